# revision 1
# baseline (speedup 1.0000x reference)
"""Trainium2 Bass kernel for KVAdapterInjector (Qwen3-style GQA attention with
LoRA-adapted virtual KV prefix).

Sharding: tensor-parallel over heads across 8 cores. Core m gets KV head m and
Q heads 4m..4m+3. Wq/Wk/Wv sharded on output dim, Wo on input dim; partial
outputs summed on host.

All heavy matmuls run fp32r (full PE rate at N>=512). Layouts keep the
contraction dim on partitions everywhere, so no on-device transposes of
hidden_states are needed (host passes hs^T, cos^T, sin^T).
"""
import sys

sys.path.insert(0, "/opt/trn_rl_repo")

import numpy as np

import concourse.bass as bass
import concourse.mybir as mybir
import concourse.tile as tile
from concourse import bacc
from concourse.bass_utils import run_bass_kernel_spmd

F32 = mybir.dt.float32
F32R = mybir.dt.float32r
AX = mybir.AxisListType
ALU = mybir.AluOpType
ACTF = mybir.ActivationFunctionType

T = 2048
D = 4096
HD = 128
NQH = 4          # q heads per core
R = 64           # virtual tokens
RANK = 16
EPS = 1e-6
SCALING = HD ** -0.5
NTC = 4          # T chunks of 512
TC = 512
ND = D // 128    # 32 contraction tiles


def build_nc():
    nc = bacc.Bacc(None, target_bir_lowering=False, debug=False)

    # ---- DRAM I/O ----
    hsT = nc.dram_tensor("hsT", (D, T), F32, kind="ExternalInput")
    wq = nc.dram_tensor("wq", (D, NQH * HD), F32, kind="ExternalInput")
    wk = nc.dram_tensor("wk", (D, HD), F32, kind="ExternalInput")
    wv = nc.dram_tensor("wv", (D, HD), F32, kind="ExternalInput")
    wo = nc.dram_tensor("wo", (NQH * HD, D), F32, kind="ExternalInput")
    vkT = nc.dram_tensor("vkT", (HD, R), F32, kind="ExternalInput")
    vvT = nc.dram_tensor("vvT", (HD, R), F32, kind="ExternalInput")
    lkA = nc.dram_tensor("lkA", (HD, RANK), F32, kind="ExternalInput")
    lkB = nc.dram_tensor("lkB", (RANK, HD), F32, kind="ExternalInput")  # pre-scaled
    lvA = nc.dram_tensor("lvA", (HD, RANK), F32, kind="ExternalInput")
    lvB = nc.dram_tensor("lvB", (RANK, HD), F32, kind="ExternalInput")  # pre-scaled
    qw = nc.dram_tensor("qw", (HD, 1), F32, kind="ExternalInput")
    kw = nc.dram_tensor("kw", (HD, 1), F32, kind="ExternalInput")
    cosT = nc.dram_tensor("cosT", (HD, T), F32, kind="ExternalInput")
    sinT = nc.dram_tensor("sinT", (HD, T), F32, kind="ExternalInput")
    maskT = nc.dram_tensor("maskT", (128, 4 * TC), F32, kind="ExternalInput")
    rotm = nc.dram_tensor("rotm", (HD, HD), F32, kind="ExternalInput")
    ident = nc.dram_tensor("ident", (128, 128), F32, kind="ExternalInput")
    onesc = nc.dram_tensor("onesc", (128, 1), F32, kind="ExternalInput")
    onesr = nc.dram_tensor("onesr", (1, 128), F32, kind="ExternalInput")
    out = nc.dram_tensor("out", (T, D), F32, kind="ExternalOutput")

    r = lambda ap: ap.bitcast(F32R)

    from contextlib import ExitStack
    with tile.TileContext(nc) as tc, ExitStack() as est:
        cp = est.enter_context(tc.tile_pool(name="consts", bufs=1))
        pp = est.enter_context(tc.tile_pool(name="persist", bufs=1))

        # ---- consts in SBUF ----
        cosT_s = cp.tile([HD, T], F32)
        sinT_s = cp.tile([HD, T], F32)
        maskT_s = cp.tile([128, 4 * TC], F32)
        rotm_s = cp.tile([HD, HD], F32R)
        ident_s = cp.tile([128, 128], F32R)
        onesc_s = cp.tile([128, 1], F32R)
        onesr_s = cp.tile([1, 128], F32R)
        qw_s = cp.tile([HD, 1], F32)
        epsc = cp.tile([128, 1], F32)
        nc.vector.memset(epsc[:], EPS)
        kw_s = cp.tile([HD, 1], F32)
        vkT_s = cp.tile([HD, R], F32R)
        vvT_s = cp.tile([HD, R], F32R)
        lkA_s = cp.tile([HD, RANK], F32R)
        lkB_s = cp.tile([RANK, HD], F32R)
        lvA_s = cp.tile([HD, RANK], F32R)
        lvB_s = cp.tile([RANK, HD], F32R)
        nc.sync.dma_start(cosT_s[:], cosT[:])
        nc.sync.dma_start(sinT_s[:], sinT[:])
        nc.sync.dma_start(maskT_s[:], maskT[:])
        nc.sync.dma_start(rotm_s[:], r(rotm[:]))
        nc.sync.dma_start(ident_s[:], r(ident[:]))
        nc.sync.dma_start(onesc_s[:], r(onesc[:]))
        nc.sync.dma_start(onesr_s[:], r(onesr[:]))
        nc.sync.dma_start(qw_s[:], qw[:])
        nc.sync.dma_start(kw_s[:], kw[:])
        nc.sync.dma_start(vkT_s[:], r(vkT[:]))
        nc.sync.dma_start(vvT_s[:], r(vvT[:]))
        nc.sync.dma_start(lkA_s[:], r(lkA[:]))
        nc.sync.dma_start(lkB_s[:], r(lkB[:]))
        nc.sync.dma_start(lvA_s[:], r(lvA[:]))
        nc.sync.dma_start(lvB_s[:], r(lvB[:]))

        # ---- persistent activations ----
        qT = [pp.tile([HD, T], F32R, tag=f"qT{h}", name=f"qT{h}") for h in range(NQH)]
        kT = pp.tile([HD, R + T], F32R)           # cols 0:64 = adapted virtual keys
        vT = pp.tile([HD, T], F32R)
        vnat = pp.tile([128, 128 + T], F32R)      # slot b at cols 128b; slot 0 virtual
        oT = qT  # alias: qT[h][:, ts] is dead after its attention chunk
        vvirtT = pp.tile([HD, R], F32R)

        # ================= Phase 1: LoRA-adapt virtual KV =================
        with tc.tile_pool(name="lora_ps", bufs=1, space="PSUM") as lps, \
             tc.tile_pool(name="lora_sb", bufs=4) as lsb:
            # keys: kT[:, 0:64] = vkT + Bk^T Ak^T vkT  (Bk pre-scaled)
            t1 = lps.tile([RANK, R], F32)
            nc.tensor.matmul(t1[:], lkA_s[:], vkT_s[:], start=True, stop=True)
            t1s = lsb.tile([RANK, R], F32R)
            nc.scalar.copy(t1s[:], t1[:])
            t2 = lps.tile([HD, R], F32)
            nc.tensor.matmul(t2[:], lkB_s[:], t1s[:], start=True, stop=True)
            nc.vector.tensor_add(kT[:, 0:R], vkT_s[:].bitcast(F32), t2[:])
            # values
            u1 = lps.tile([RANK, R], F32)
            nc.tensor.matmul(u1[:], lvA_s[:], vvT_s[:], start=True, stop=True)
            u1s = lsb.tile([RANK, R], F32R)
            nc.scalar.copy(u1s[:], u1[:])
            u2 = lps.tile([HD, R], F32)
            nc.tensor.matmul(u2[:], lvB_s[:], u1s[:], start=True, stop=True)
            nc.vector.tensor_add(vvirtT[:], vvT_s[:].bitcast(F32), u2[:])
            # transpose virtual values to natural layout -> vnat[0:64, 0:128]
            vtp = lps.tile([R, HD], F32R)
            nc.tensor.transpose(vtp[:], vvirtT[:], ident_s[:])
            nc.scalar.copy(vnat[0:R, 0:128], vtp[:])

        # ================= Phase 2: QKV projections =================
        with tc.tile_pool(name="wpool", bufs=1) as wp, \
             tc.tile_pool(name="hstream", bufs=8) as hsp, \
             tc.tile_pool(name="proj_ps", bufs=1, space="PSUM") as prps:
            wq_s = wp.tile([128, ND, NQH * HD], F32R)
            wk_s = wp.tile([128, ND, HD], F32R)
            wv_s = wp.tile([128, ND, HD], F32R)
            for d in range(ND):
                nc.sync.dma_start(wq_s[:, d, :], r(wq[d * 128:(d + 1) * 128, :]))
                nc.sync.dma_start(wk_s[:, d, :], r(wk[d * 128:(d + 1) * 128, :]))
                nc.sync.dma_start(wv_s[:, d, :], r(wv[d * 128:(d + 1) * 128, :]))
            for tcj in range(NTC):
                ts = slice(tcj * TC, (tcj + 1) * TC)
                pq = [prps.tile([128, TC], F32, tag=f"pq{h}", name=f"pq{h}") for h in range(NQH)]
                pk = prps.tile([128, TC], F32, tag="pk")
                pv = prps.tile([128, TC], F32, tag="pv")
                for d in range(ND):
                    hs_d = hsp.tile([128, TC], F32R)
                    nc.sync.dma_start(hs_d[:], r(hsT[d * 128:(d + 1) * 128, ts]))
                    st, sp = (d == 0), (d == ND - 1)
                    for h in range(NQH):
                        nc.tensor.matmul(pq[h][:], wq_s[:, d, h * HD:(h + 1) * HD],
                                         hs_d[:], start=st, stop=sp)
                    nc.tensor.matmul(pk[:], wk_s[:, d, :], hs_d[:], start=st, stop=sp)
                    nc.tensor.matmul(pv[:], wv_s[:, d, :], hs_d[:], start=st, stop=sp)
                for h in range(NQH):
                    nc.scalar.copy(qT[h][:, ts], pq[h][:])
                nc.scalar.copy(kT[:, R + tcj * TC: R + (tcj + 1) * TC], pk[:])
                nc.scalar.copy(vT[:, ts], pv[:])

        # ============ Phase 3: per-head RMSNorm + RoPE on q, k ============
        with tc.tile_pool(name="nrm_ps", bufs=2, space="PSUM") as nps, \
             tc.tile_pool(name="nrm_sb", bufs=2) as nsb:
            targets = [(qT[h], qw_s) for h in range(NQH)] + [(None, kw_s)]
            for xT, w in targets:
                get = (lambda a, b: xT[:, a:b]) if xT is not None else \
                      (lambda a, b: kT[:, R + a: R + b])
                for j in range(NTC):
                    a, b = j * TC, (j + 1) * TC
                    sq = nsb.tile([HD, TC], F32R, tag="sq", bufs=3)
                    nc.scalar.square(sq[:], get(a, b).bitcast(F32))
                    ssp = nps.tile([1, TC], F32, tag="ss")
                    nc.tensor.matmul(ssp[:], onesc_s[:], sq[:],
                                     start=True, stop=True)
                    srt = nsb.tile([1, TC], F32, tag="srt")
                    nc.scalar.activation(srt[:], ssp[:], ACTF.Sqrt,
                                         bias=epsc[0:1, :], scale=1.0 / HD)
                    rinv = nsb.tile([1, TC], F32R, tag="rinv")
                    with nc.allow_low_precision(reason="f32r same width as f32"):
                        nc.vector.reciprocal(rinv[:], srt[:])
                    rb = nps.tile([128, TC], F32, tag="rb")
                    nc.tensor.matmul(rb[:], onesr_s[:], rinv[:],
                                     start=True, stop=True)
                    xn = nsb.tile([HD, TC], F32R, tag="xn")
                    nc.vector.scalar_tensor_tensor(
                        xn[:], get(a, b).bitcast(F32), w[:], rb[:],
                        op0=ALU.mult, op1=ALU.mult)
                    pr = nps.tile([HD, TC], F32, tag="pr")
                    nc.tensor.matmul(pr[:], rotm_s[:], xn[:], start=True, stop=True)
                    t1 = nsb.tile([HD, TC], F32, tag="t1")
                    nc.vector.tensor_mul(t1[:], xn[:].bitcast(F32), cosT_s[:, a:b])
                    t2 = nsb.tile([HD, TC], F32, tag="t2")
                    nc.vector.tensor_mul(t2[:], pr[:], sinT_s[:, a:b])
                    nc.vector.tensor_add(get(a, b), t1[:], t2[:])

        # ============ Phase 4: transpose V to natural layout ============
        with tc.tile_pool(name="vt_ps", bufs=4, space="PSUM") as vps:
            for b in range(T // 128):
                pt = vps.tile([128, 128], F32R)
                nc.tensor.transpose(pt[:], vT[:, b * 128:(b + 1) * 128], ident_s[:])
                nc.scalar.copy(vnat[:, (b + 1) * 128:(b + 2) * 128], pt[:])

        # ================= Phase 5: attention =================
        with tc.tile_pool(name="at_ps", bufs=1, space="PSUM") as aps, \
             tc.tile_pool(name="at_acc", bufs=2, space="PSUM") as accps, \
             tc.tile_pool(name="at_sum", bufs=2, space="PSUM") as sumps, \
             tc.tile_pool(name="at_sb", bufs=4) as asb:
            for tcj in range(NTC):
                for h in range(NQH):
                    ts = slice(tcj * TC, (tcj + 1) * TC)
                    nreal = 4 * tcj + 4
                    po = accps.tile([128, TC], F32, tag="po")
                    psum = sumps.tile([1, TC], F32, tag="ps")
                    nblk = nreal + 1
                    for i in range(nblk):
                        virt = (i == 0)
                        bb = i - 1
                        rows = R if virt else 128
                        st_ = aps.tile([128, TC], F32, tag="st", bufs=3)
                        if virt:
                            lhs = kT[:, 0:R]
                        else:
                            lhs = kT[:, R + bb * 128: R + (bb + 1) * 128]
                        nc.tensor.matmul(st_[:rows, :], lhs, qT[h][:, ts],
                                         start=True, stop=True)
                        if (not virt) and bb >= 4 * tcj:
                            j = bb - 4 * tcj
                            nc.vector.tensor_add(
                                st_[:], st_[:], maskT_s[:, j * TC:(j + 1) * TC])
                        pe = asb.tile([128, TC], F32R, tag="pe", bufs=6)
                        nc.scalar.activation(pe[:rows, :], st_[:rows, :], ACTF.Exp,
                                             scale=SCALING)
                        nc.tensor.matmul(psum[:], onesc_s[:rows, :], pe[:rows, :],
                                         start=(i == 0), stop=(i == nblk - 1))
                        if virt:
                            vsl = vnat[0:R, 0:128]
                        else:
                            vsl = vnat[:, (bb + 1) * 128:(bb + 2) * 128]
                        nc.tensor.matmul(po[:], vsl, pe[:rows, :],
                                         start=(i == 0), stop=(i == nblk - 1))
                    rinv = asb.tile([1, TC], F32R, tag="arinv")
                    with nc.allow_low_precision(reason="f32r same width as f32"):
                        nc.vector.reciprocal(rinv[:], psum[:])
                    rb = aps.tile([128, TC], F32, tag="arb", bufs=1)
                    nc.tensor.matmul(rb[:], onesr_s[:], rinv[:], start=True, stop=True)
                    rbs = asb.tile([128, TC], F32, tag="rbs")
                    nc.scalar.copy(rbs[:], rb[:])
                    nc.vector.tensor_mul(oT[h][:, ts], po[:], rbs[:])

        # ================= Phase 6: output projection =================
        with tc.tile_pool(name="op_ps", bufs=4, space="PSUM") as ops, \
             tc.tile_pool(name="wo_sb", bufs=2) as wosb, \
             tc.tile_pool(name="out_sb", bufs=4) as outsb:
            for j2 in range(D // TC):
                wo_t = [wosb.tile([128, TC], F32R, tag=f"wo{h}", name=f"wo{h}")
                        for h in range(NQH)]
                for h in range(NQH):
                    nc.sync.dma_start(
                        wo_t[h][:],
                        r(wo[h * HD:(h + 1) * HD, j2 * TC:(j2 + 1) * TC]))
                for tt in range(T // 128):
                    po = ops.tile([128, TC], F32, tag="opo")
                    for h in range(NQH):
                        nc.tensor.matmul(po[:], oT[h][:, tt * 128:(tt + 1) * 128],
                                         wo_t[h][:], start=(h == 0), stop=(h == NQH - 1))
                    ob = outsb.tile([128, TC], F32, tag="ob")
                    nc.scalar.copy(ob[:], po[:])
                    nc.sync.dma_start(
                        out[tt * 128:(tt + 1) * 128, j2 * TC:(j2 + 1) * TC], ob[:])

    nc.compile()
    return nc


_NC_CACHE = {}


def _get_nc():
    if "nc" not in _NC_CACHE:
        _NC_CACHE["nc"] = build_nc()
    return _NC_CACHE["nc"]


def kernel(**inputs) -> np.ndarray:
    f = lambda k: np.asarray(inputs[k], np.float32)
    hs = f("hidden_states")[0]            # (T, D)
    vk = f("virtual_keys")[0]             # (HKV, R, HD)
    vv = f("virtual_values")[0]
    Wq, Wk, Wv, Wo = f("Wq"), f("Wk"), f("Wv"), f("Wo")
    qnw, knw = f("q_norm_w"), f("k_norm_w")
    lkA, lkB = f("lora_k_A"), f("lora_k_B")
    lvA, lvB = f("lora_v_A"), f("lora_v_B")
    sk = np.float32(np.asarray(inputs["scale_k"]))
    sv = np.float32(np.asarray(inputs["scale_v"]))
    am = f("attention_mask")              # (1,1,T,T)
    cos, sin = f("cos"), f("sin")         # (T, HD)

    hsT = np.ascontiguousarray(hs.T)
    cosT = np.ascontiguousarray(cos.T)
    sinT = np.ascontiguousarray(sin.T)
    # diagonal causal mask blocks, transposed: block j = am[0,0,0:512,128j:+128].T
    maskT = np.ascontiguousarray(
        np.concatenate([am[0, 0, 0:TC, 128 * j:128 * (j + 1)].T for j in range(4)],
                       axis=1))
    rotm = np.zeros((HD, HD), np.float32)
    for dd in range(64):
        rotm[dd + 64, dd] = -1.0          # rot[d] = -x[d+64], d<64
        rotm[dd, dd + 64] = 1.0           # rot[d] = +x[d-64], d>=64
    ident = np.eye(128, dtype=np.float32)
    onesc = np.ones((128, 1), np.float32)
    onesr = np.ones((1, 128), np.float32)
    lkBs = np.ascontiguousarray(lkB * sk)
    lvBs = np.ascontiguousarray(lvB * sv)

    in_maps = []
    for m in range(8):
        in_maps.append({
            "hsT": hsT,
            "wq": np.ascontiguousarray(Wq[:, 512 * m:512 * (m + 1)]),
            "wk": np.ascontiguousarray(Wk[:, 128 * m:128 * (m + 1)]),
            "wv": np.ascontiguousarray(Wv[:, 128 * m:128 * (m + 1)]),
            "wo": np.ascontiguousarray(Wo[512 * m:512 * (m + 1), :]),
            "vkT": np.ascontiguousarray(vk[m].T),
            "vvT": np.ascontiguousarray(vv[m].T),
            "lkA": lkA, "lkB": lkBs, "lvA": lvA, "lvB": lvBs,
            "qw": np.ascontiguousarray(qnw[:, None]),
            "kw": np.ascontiguousarray(knw[:, None]),
            "cosT": cosT, "sinT": sinT, "maskT": maskT,
            "rotm": rotm, "ident": ident, "onesc": onesc, "onesr": onesr,
        })

    nc = _get_nc()
    res = run_bass_kernel_spmd(nc, in_maps, core_ids=list(range(8)))
    acc = res.results[0]["out"].astype(np.float32)
    for m in range(1, 8):
        acc = acc + res.results[m]["out"]
    return acc[None]  # (1, T, D)



# revision 3
# speedup vs baseline: 1.0449x; 1.0449x over previous
"""Trainium2 Bass kernel for KVAdapterInjector (Qwen3-style GQA attention with
LoRA-adapted virtual KV prefix).

Sharding: tensor-parallel over heads across 8 cores. Core m gets KV head m and
Q heads 4m..4m+3. Wq/Wk/Wv sharded on output dim, Wo on input dim; partial
outputs summed on host.

v2 layout/scheduling notes:
- hs/Wq/Wk/Wv/Wo/out in bf16 (halves DMA; matmul rate identical to f32r).
- V is projected directly into natural [token, HD] layout (no transposes).
- Softmax denominators accumulated on DVE; rsqrt for RMSNorm computed as
  exp(-0.5*ln(x)) on Act so the whole kernel uses one activation table.
- Causal diagonal blocks are windowed (masked columns not computed); only a
  [128,128] triangle mask remains.
- Flat PSUM pools (2+2+2+2 banks) and interleaved emission so the tile
  scheduler overlaps projection / norm+rope / attention / out-projection.
"""
import sys

sys.path.insert(0, "/opt/trn_rl_repo")

import numpy as np
import ml_dtypes

import concourse.bass as bass
import concourse.mybir as mybir
import concourse.tile as tile
from concourse import bacc
from concourse.bass_utils import run_bass_kernel_spmd

F32 = mybir.dt.float32
F32R = mybir.dt.float32r
BF16 = mybir.dt.bfloat16
ALU = mybir.AluOpType
ACTF = mybir.ActivationFunctionType

T = 2048
D = 4096
HD = 128
NQH = 4          # q heads per core
R = 64           # virtual tokens
RANK = 16
EPS = 1e-6
SCALING = HD ** -0.5
ND = D // 128    # 32 contraction tiles
TC = 256         # projection T-chunk
NPC = T // TC    # 8 projection chunks
AC = 512         # attention / norm T-chunk
NAC = T // AC    # 4 attention chunks


def build_nc():
    nc = bacc.Bacc(None, target_bir_lowering=False, debug=False)

    # ---- DRAM I/O ----
    hsT = nc.dram_tensor("hsT", (D, T), BF16, kind="ExternalInput")
    wq = nc.dram_tensor("wq", (D, NQH * HD), BF16, kind="ExternalInput")
    wkv = nc.dram_tensor("wkv", (D, 2 * HD), BF16, kind="ExternalInput")
    wo = nc.dram_tensor("wo", (NQH * HD, D), BF16, kind="ExternalInput")
    vkT = nc.dram_tensor("vkT", (HD, R), F32, kind="ExternalInput")
    vvT = nc.dram_tensor("vvT", (HD, R), F32, kind="ExternalInput")
    lkA = nc.dram_tensor("lkA", (HD, RANK), F32, kind="ExternalInput")
    lkB = nc.dram_tensor("lkB", (RANK, HD), F32, kind="ExternalInput")  # pre-scaled
    lvA = nc.dram_tensor("lvA", (HD, RANK), F32, kind="ExternalInput")
    lvB = nc.dram_tensor("lvB", (RANK, HD), F32, kind="ExternalInput")  # pre-scaled
    qw = nc.dram_tensor("qw", (HD, 1), F32, kind="ExternalInput")
    kw = nc.dram_tensor("kw", (HD, 1), F32, kind="ExternalInput")
    cosT = nc.dram_tensor("cosT", (HD, T), F32, kind="ExternalInput")
    sinT = nc.dram_tensor("sinT", (HD, T), F32, kind="ExternalInput")
    mtri = nc.dram_tensor("mtri", (128, 128), F32, kind="ExternalInput")
    rotm = nc.dram_tensor("rotm", (HD, HD), F32, kind="ExternalInput")
    ident = nc.dram_tensor("ident", (128, 128), F32, kind="ExternalInput")
    onesc = nc.dram_tensor("onesc", (128, 1), F32, kind="ExternalInput")
    onesr = nc.dram_tensor("onesr", (1, 128), F32, kind="ExternalInput")
    out = nc.dram_tensor("out", (T, D), BF16, kind="ExternalOutput")

    r = lambda ap: ap.bitcast(F32R)

    from contextlib import ExitStack
    with tile.TileContext(nc) as tc, ExitStack() as est:
        cp = est.enter_context(tc.tile_pool(name="consts", bufs=1))
        pp = est.enter_context(tc.tile_pool(name="persist", bufs=1))
        # PSUM pools: 2+2+2+2 = 8 banks
        paccp = est.enter_context(tc.tile_pool(name="pacc", bufs=2, space="PSUM"))
        stp = est.enter_context(tc.tile_pool(name="stp", bufs=2, space="PSUM"))
        pop = est.enter_context(tc.tile_pool(name="pop", bufs=2, space="PSUM"))
        auxp = est.enter_context(tc.tile_pool(name="auxp", bufs=2, space="PSUM"))
        # SBUF streaming pools
        hsp = est.enter_context(tc.tile_pool(name="hsp", bufs=2))
        pep = est.enter_context(tc.tile_pool(name="pep", bufs=3))
        accp = est.enter_context(tc.tile_pool(name="accp", bufs=2))
        nrm = est.enter_context(tc.tile_pool(name="nrm", bufs=2))
        ostp = est.enter_context(tc.tile_pool(name="ostp", bufs=2))

        # ---- small consts (emitted first: cheap DMAs, needed early) ----
        vkT_s = cp.tile([HD, R], F32R)
        vvT_s = cp.tile([HD, R], F32R)
        lkA_s = cp.tile([HD, RANK], F32R)
        lkB_s = cp.tile([RANK, HD], F32R)
        lvA_s = cp.tile([HD, RANK], F32R)
        lvB_s = cp.tile([RANK, HD], F32R)
        onesc_s = cp.tile([128, 1], F32R)
        onesr_s = cp.tile([1, 128], F32R)
        qw_s = cp.tile([HD, 1], F32)
        kw_s = cp.tile([HD, 1], F32)
        mtri_s = cp.tile([128, 128], F32)
        rotm_s = cp.tile([HD, HD], F32R)
        ident_s = cp.tile([128, 128], F32R)
        epsc = cp.tile([128, 1], F32)
        nc.vector.memset(epsc[:], EPS)
        nc.sync.dma_start(vkT_s[:], r(vkT[:]))
        nc.sync.dma_start(vvT_s[:], r(vvT[:]))
        nc.sync.dma_start(lkA_s[:], r(lkA[:]))
        nc.sync.dma_start(lkB_s[:], r(lkB[:]))
        nc.sync.dma_start(lvA_s[:], r(lvA[:]))
        nc.sync.dma_start(lvB_s[:], r(lvB[:]))
        nc.sync.dma_start(onesc_s[:], r(onesc[:]))
        nc.sync.dma_start(onesr_s[:], r(onesr[:]))
        nc.sync.dma_start(qw_s[:], qw[:])
        nc.sync.dma_start(kw_s[:], kw[:])
        nc.sync.dma_start(mtri_s[:], mtri[:])
        nc.sync.dma_start(rotm_s[:], r(rotm[:]))
        nc.sync.dma_start(ident_s[:], r(ident[:]))

        # ---- big persistent tensors ----
        wq_s = pp.tile([128, ND, NQH * HD], BF16)
        wkv_s = pp.tile([128, ND, 2 * HD], BF16)
        wo_s = pp.tile([128, NQH, D], BF16)
        qT = [pp.tile([HD, T], F32, tag=f"qT{h}", name=f"qT{h}") for h in range(NQH)]
        kT = pp.tile([HD, R + T], F32)
        vnat = pp.tile([128, 128 + T], F32)   # cols 0:128 rows 0:64 = virtual V
        cosT_s = cp.tile([HD, T], F32)
        sinT_s = cp.tile([HD, T], F32)

        # hs chunk prefetch ring
        hs_tiles = [None] * NPC

        def hs_fetch(pc):
            t_ = hsp.tile([128, ND, TC], BF16, tag="hs")
            nc.sync.dma_start(
                t_[:],
                hsT[:, pc * TC:(pc + 1) * TC].rearrange("(d p) t -> p d t", p=128))
            hs_tiles[pc] = t_

        # weight / hs DMA order = SP priority order
        nc.sync.dma_start(
            wq_s[:, :, 0:256], wq[:, 0:256].rearrange("(d p) c -> p d c", p=128))
        hs_fetch(0)
        nc.sync.dma_start(
            wq_s[:, :, 256:512], wq[:, 256:512].rearrange("(d p) c -> p d c", p=128))
        nc.sync.dma_start(wkv_s[:], wkv[:].rearrange("(d p) c -> p d c", p=128))
        hs_fetch(1)
        nc.sync.dma_start(cosT_s[:], cosT[:])
        nc.sync.dma_start(sinT_s[:], sinT[:])
        nc.sync.dma_start(wo_s[:], wo[:].rearrange("(h p) c -> p h c", p=128))

        # ================= LoRA-adapt virtual KV =================
        vvirtT = cp.tile([HD, R], F32)
        t1 = auxp.tile([128, 512], F32, tag="aux")
        nc.tensor.matmul(t1[0:RANK, 0:R], lkA_s[:], vkT_s[:], start=True, stop=True)
        t1s = cp.tile([RANK, R], F32R)
        nc.scalar.copy(t1s[:], t1[0:RANK, 0:R])
        t2 = auxp.tile([128, 512], F32, tag="aux")
        nc.tensor.matmul(t2[0:HD, 0:R], lkB_s[:], t1s[:], start=True, stop=True)
        nc.vector.tensor_add(kT[:, 0:R], vkT_s[:].bitcast(F32), t2[0:HD, 0:R])
        u1 = auxp.tile([128, 512], F32, tag="aux")
        nc.tensor.matmul(u1[0:RANK, 0:R], lvA_s[:], vvT_s[:], start=True, stop=True)
        u1s = cp.tile([RANK, R], F32R)
        nc.scalar.copy(u1s[:], u1[0:RANK, 0:R])
        u2 = auxp.tile([128, 512], F32, tag="aux")
        nc.tensor.matmul(u2[0:HD, 0:R], lvB_s[:], u1s[:], start=True, stop=True)
        nc.vector.tensor_add(vvirtT[:], vvT_s[:].bitcast(F32), u2[0:HD, 0:R])
        vtp = auxp.tile([128, 512], F32, tag="aux")
        nc.tensor.transpose(vtp[0:R, 0:HD].bitcast(F32R), r(vvirtT[:]), ident_s[:])
        nc.gpsimd.tensor_copy(vnat[0:R, 0:128], vtp[0:R, 0:HD])

        # ================= emission helpers =================
        def proj_chunk(pc):
            if pc + 2 < NPC:
                hs_fetch(pc + 2)
            hs_t = hs_tiles[pc]
            cs = pc * TC
            # q heads
            for h in range(NQH):
                p = paccp.tile([128, TC], F32, tag="pacc")
                for d in range(ND):
                    nc.tensor.matmul(p[:], wq_s[:, d, h * HD:(h + 1) * HD],
                                     hs_t[:, d, :], start=(d == 0), stop=(d == ND - 1))
                nc.gpsimd.tensor_copy(qT[h][:, cs:cs + TC], p[:])
            # k
            p = paccp.tile([128, TC], F32, tag="pacc")
            for d in range(ND):
                nc.tensor.matmul(p[:], wkv_s[:, d, 0:HD], hs_t[:, d, :],
                                 start=(d == 0), stop=(d == ND - 1))
            nc.gpsimd.tensor_copy(kT[:, R + cs:R + cs + TC], p[:])
            # v natural: two 128-token row blocks
            for vb in range(TC // 128):
                p = paccp.tile([128, TC], F32, tag="pacc")
                for d in range(ND):
                    nc.tensor.matmul(p[:, 0:HD], hs_t[:, d, vb * 128:(vb + 1) * 128],
                                     wkv_s[:, d, HD:2 * HD],
                                     start=(d == 0), stop=(d == ND - 1))
                bg = (cs + vb * 128) // 128
                nc.gpsimd.tensor_copy(vnat[:, (bg + 1) * 128:(bg + 2) * 128],
                                      p[:, 0:HD])

        def norm_chunk(ncx):
            a, b = ncx * AC, (ncx + 1) * AC
            targets = [(qT[h][:, a:b], qw_s) for h in range(NQH)] + \
                      [(kT[:, R + a:R + b], kw_s)]
            for xap, w in targets:
                sq = nrm.tile([HD, AC], F32R, tag="sqt")
                nc.scalar.square(sq[:], xap)
                ssl = auxp.tile([128, AC], F32, tag="aux")
                nc.tensor.matmul(ssl[0:1, :], onesc_s[:], sq[:], start=True, stop=True)
                lnm = nrm.tile([1, AC], F32, tag="lnm", bufs=1)
                nc.scalar.activation(lnm[:], ssl[0:1, :], ACTF.Ln,
                                     bias=epsc[0:1, :], scale=1.0 / HD)
                rin = nrm.tile([1, AC], F32R, tag="rin", bufs=1)
                nc.scalar.activation(rin[:], lnm[:], ACTF.Exp, scale=-0.5)
                nrb = auxp.tile([128, AC], F32, tag="aux")
                nc.tensor.matmul(nrb[:], onesr_s[:], rin[:], start=True, stop=True)
                xn = nrm.tile([HD, AC], F32R, tag="xn")
                nc.vector.scalar_tensor_tensor(xn[:], xap, w[:], nrb[:],
                                               op0=ALU.mult, op1=ALU.mult)
                pr = auxp.tile([128, AC], F32, tag="aux")
                nc.tensor.matmul(pr[:], rotm_s[:], xn[:], start=True, stop=True)
                # xn <- xn * cos (in place, after pr consumed xn)
                nc.gpsimd.tensor_mul(xn[:].bitcast(F32), xn[:].bitcast(F32),
                                     cosT_s[:, a:b])
                t2_ = nrm.tile([HD, AC], F32R, tag="sqt")
                nc.vector.tensor_mul(t2_[:].bitcast(F32), pr[:], sinT_s[:, a:b])
                nc.gpsimd.tensor_add(xap, xn[:].bitcast(F32), t2_[:].bitcast(F32))

        def att_head(tcj, h):
            cs = tcj * AC
            hq = qT[h]
            acc_t = accp.tile([128, AC], F32, tag="acc")
            po_t = pop.tile([128, AC], F32, tag="po")
            nreal = 4 * tcj + 4
            for b_ in range(nreal):
                diag = b_ >= 4 * tcj
                off = 128 * (b_ - 4 * tcj) if diag else 0
                n = AC - off
                st_t = stp.tile([128, AC], F32, tag="st")
                nc.tensor.matmul(st_t[:, off:AC],
                                 r(kT[:, R + b_ * 128:R + (b_ + 1) * 128]),
                                 r(hq[:, cs + off:cs + AC]), start=True, stop=True)
                if diag:
                    nc.vector.tensor_add(st_t[:, off:off + 128],
                                         st_t[:, off:off + 128], mtri_s[:])
                pe_t = pep.tile([128, AC], F32R, tag="pe")
                nc.scalar.activation(pe_t[:, off:AC], st_t[:, off:AC], ACTF.Exp,
                                     scale=SCALING)
                if b_ == 0:
                    nc.gpsimd.tensor_copy(acc_t[:], pe_t[:].bitcast(F32))
                else:
                    nc.vector.tensor_add(acc_t[:, off:AC], acc_t[:, off:AC],
                                         pe_t[:, off:AC].bitcast(F32))
                nc.tensor.matmul(po_t[:, off:AC],
                                 r(vnat[:, (b_ + 1) * 128:(b_ + 2) * 128]),
                                 pe_t[:, off:AC], start=(b_ == 0), stop=False)
            # virtual prefix block (full width, 64 rows)
            st_t = stp.tile([128, AC], F32, tag="st")
            nc.tensor.matmul(st_t[0:R, :], r(kT[:, 0:R]), r(hq[:, cs:cs + AC]),
                             start=True, stop=True)
            pe_t = pep.tile([128, AC], F32R, tag="pe")
            nc.scalar.activation(pe_t[0:R, :], st_t[0:R, :], ACTF.Exp, scale=SCALING)
            nc.vector.tensor_add(acc_t[0:R, :], acc_t[0:R, :],
                                 pe_t[0:R, :].bitcast(F32))
            nc.tensor.matmul(po_t[:], r(vnat[0:R, 0:128]), pe_t[0:R, :],
                             start=False, stop=True)
            # normalize: oT (bf16, aliased into qT storage) = po / den
            den = auxp.tile([128, AC], F32, tag="aux")
            nc.tensor.matmul(den[0:1, :], onesc_s[:], r(acc_t[:]),
                             start=True, stop=True)
            ari = nrm.tile([1, AC], F32R, tag="ari", bufs=1)
            with nc.allow_low_precision(reason="f32r same width as f32"):
                nc.vector.reciprocal(ari[:], den[0:1, :])
            rb = auxp.tile([128, AC], F32, tag="aux")
            nc.tensor.matmul(rb[:], onesr_s[:], ari[:], start=True, stop=True)
            oTv = hq[:].bitcast(BF16)   # [128, 2*T] bf16; cols 0:T = oT
            nc.vector.tensor_mul(oTv[:, cs:cs + AC], po_t[:], rb[:])

        def out_chunk(c):
            for tt in range(4 * c, 4 * c + 4):
                for j2 in range(D // 512):
                    op = auxp.tile([128, 512], F32, tag="aux")
                    for h in range(NQH):
                        oTv = qT[h][:].bitcast(BF16)
                        nc.tensor.matmul(op[:], oTv[:, tt * 128:(tt + 1) * 128],
                                         wo_s[:, h, j2 * 512:(j2 + 1) * 512],
                                         start=(h == 0), stop=(h == NQH - 1))
                    ost = ostp.tile([128, 512], BF16, tag="ost")
                    if (tt + j2) % 2 == 0:
                        nc.scalar.copy(ost[:], op[:])
                    else:
                        nc.vector.tensor_copy(ost[:], op[:])
                    nc.sync.dma_start(
                        out[tt * 128:(tt + 1) * 128, j2 * 512:(j2 + 1) * 512],
                        ost[:])

        # ================= master emission sequence =================
        proj_chunk(0)
        proj_chunk(1)
        norm_chunk(0)
        proj_chunk(2)
        proj_chunk(3)
        for h in range(NQH):
            att_head(0, h)
        norm_chunk(1)
        proj_chunk(4)
        proj_chunk(5)
        for h in range(NQH):
            att_head(1, h)
        out_chunk(0)
        norm_chunk(2)
        proj_chunk(6)
        proj_chunk(7)
        for h in range(NQH):
            att_head(2, h)
        out_chunk(1)
        norm_chunk(3)
        for h in range(NQH):
            att_head(3, h)
        out_chunk(2)
        out_chunk(3)

    nc.compile()
    return nc


_NC_CACHE = {}


def _get_nc():
    if "nc" not in _NC_CACHE:
        _NC_CACHE["nc"] = build_nc()
    return _NC_CACHE["nc"]


def kernel(**inputs) -> np.ndarray:
    f = lambda k: np.asarray(inputs[k], np.float32)
    bf = lambda a: np.ascontiguousarray(a).astype(ml_dtypes.bfloat16)
    hs = f("hidden_states")[0]            # (T, D)
    vk = f("virtual_keys")[0]             # (HKV, R, HD)
    vv = f("virtual_values")[0]
    Wq, Wk, Wv, Wo = f("Wq"), f("Wk"), f("Wv"), f("Wo")
    qnw, knw = f("q_norm_w"), f("k_norm_w")
    lkA, lkB = f("lora_k_A"), f("lora_k_B")
    lvA, lvB = f("lora_v_A"), f("lora_v_B")
    sk = np.float32(np.asarray(inputs["scale_k"]))
    sv = np.float32(np.asarray(inputs["scale_v"]))
    am = f("attention_mask")              # (1,1,T,T)
    cos, sin = f("cos"), f("sin")         # (T, HD)

    hsT = bf(hs.T)
    cosT = np.ascontiguousarray(cos.T)
    sinT = np.ascontiguousarray(sin.T)
    # aligned [128,128] causal triangle: rows k, cols q, masked iff k > q
    mtri = np.ascontiguousarray(am[0, 0, 0:128, 0:128].T)
    rotm = np.zeros((HD, HD), np.float32)
    for dd in range(64):
        rotm[dd + 64, dd] = -1.0          # rot[d] = -x[d+64], d<64
        rotm[dd, dd + 64] = 1.0           # rot[d] = +x[d-64], d>=64
    ident = np.eye(128, dtype=np.float32)
    onesc = np.ones((128, 1), np.float32)
    onesr = np.ones((1, 128), np.float32)
    lkBs = np.ascontiguousarray(lkB * sk)
    lvBs = np.ascontiguousarray(lvB * sv)

    in_maps = []
    for m in range(8):
        in_maps.append({
            "hsT": hsT,
            "wq": bf(Wq[:, 512 * m:512 * (m + 1)]),
            "wkv": bf(np.concatenate(
                [Wk[:, 128 * m:128 * (m + 1)], Wv[:, 128 * m:128 * (m + 1)]],
                axis=1)),
            "wo": bf(Wo[512 * m:512 * (m + 1), :]),
            "vkT": np.ascontiguousarray(vk[m].T),
            "vvT": np.ascontiguousarray(vv[m].T),
            "lkA": lkA, "lkB": lkBs, "lvA": lvA, "lvB": lvBs,
            "qw": np.ascontiguousarray(qnw[:, None]),
            "kw": np.ascontiguousarray(knw[:, None]),
            "cosT": cosT, "sinT": sinT, "mtri": mtri,
            "rotm": rotm, "ident": ident, "onesc": onesc, "onesr": onesr,
        })

    nc = _get_nc()
    res = run_bass_kernel_spmd(nc, in_maps, core_ids=list(range(8)))
    acc = res.results[0]["out"].astype(np.float32)
    for m in range(1, 8):
        acc = acc + res.results[m]["out"].astype(np.float32)
    return acc[None]  # (1, T, D)


# revision 7
# speedup vs baseline: 1.2906x; 1.2351x over previous
"""Trainium2 Bass kernel for KVAdapterInjector (Qwen3-style GQA attention with
LoRA-adapted virtual KV prefix).

Sharding: tensor-parallel over heads across 8 cores. Core m gets KV head m and
Q heads 4m..4m+3. Wq/Wk/Wv sharded on output dim, Wo on input dim; partial
outputs summed on host.

v2 layout/scheduling notes:
- hs/Wq/Wk/Wv/Wo/out in bf16 (halves DMA; matmul rate identical to f32r).
- V is projected directly into natural [token, HD] layout (no transposes).
- Softmax denominators accumulated on DVE; rsqrt for RMSNorm computed as
  exp(-0.5*ln(x)) on Act so the whole kernel uses one activation table.
- Causal diagonal blocks are windowed (masked columns not computed); only a
  [128,128] triangle mask remains.
- Flat PSUM pools (2+2+2+2 banks) and interleaved emission so the tile
  scheduler overlaps projection / norm+rope / attention / out-projection.
"""
import sys

sys.path.insert(0, "/opt/trn_rl_repo")

import numpy as np
import ml_dtypes

import concourse.bass as bass
import concourse.mybir as mybir
import concourse.tile as tile
from concourse import bacc
from concourse.bass_utils import run_bass_kernel_spmd

F32 = mybir.dt.float32
F32R = mybir.dt.float32r
BF16 = mybir.dt.bfloat16
ALU = mybir.AluOpType
ACTF = mybir.ActivationFunctionType

T = 2048
D = 4096
HD = 128
NQH = 4          # q heads per core
R = 64           # virtual tokens
RANK = 16
EPS = 1e-6
SCALING = HD ** -0.5
ND = D // 128    # 32 contraction tiles
TC = 256         # projection T-chunk
NPC = T // TC    # 8 projection chunks
AC = 512         # attention / norm T-chunk
NAC = T // AC    # 4 attention chunks


def build_nc():
    nc = bacc.Bacc(None, target_bir_lowering=False, debug=False)

    # ---- DRAM I/O ----
    hsT = nc.dram_tensor("hsT", (D, T), BF16, kind="ExternalInput")
    wq = nc.dram_tensor("wq", (D, NQH * HD), BF16, kind="ExternalInput")
    wkv = nc.dram_tensor("wkv", (D, 2 * HD), BF16, kind="ExternalInput")
    wo = nc.dram_tensor("wo", (NQH * HD, D), BF16, kind="ExternalInput")
    vkT = nc.dram_tensor("vkT", (HD, R), F32, kind="ExternalInput")
    vvT = nc.dram_tensor("vvT", (HD, R), F32, kind="ExternalInput")
    lkA = nc.dram_tensor("lkA", (HD, RANK), F32, kind="ExternalInput")
    lkB = nc.dram_tensor("lkB", (RANK, HD), F32, kind="ExternalInput")  # pre-scaled
    lvA = nc.dram_tensor("lvA", (HD, RANK), F32, kind="ExternalInput")
    lvB = nc.dram_tensor("lvB", (RANK, HD), F32, kind="ExternalInput")  # pre-scaled
    qw = nc.dram_tensor("qw", (HD, 1), F32, kind="ExternalInput")
    kw = nc.dram_tensor("kw", (HD, 1), F32, kind="ExternalInput")
    cosT = nc.dram_tensor("cosT", (HD, T), F32, kind="ExternalInput")
    sinT = nc.dram_tensor("sinT", (HD, T), F32, kind="ExternalInput")
    mtri = nc.dram_tensor("mtri", (128, 128), F32, kind="ExternalInput")
    rotm = nc.dram_tensor("rotm", (HD, HD), F32, kind="ExternalInput")
    ident = nc.dram_tensor("ident", (128, 128), F32, kind="ExternalInput")
    onesc = nc.dram_tensor("onesc", (128, 1), F32, kind="ExternalInput")
    onesr = nc.dram_tensor("onesr", (1, 128), F32, kind="ExternalInput")
    onesel = nc.dram_tensor("onesel", (128, 100), F32, kind="ExternalInput")
    onesel2 = nc.dram_tensor("onesel2", (10, 1280), F32, kind="ExternalInput")
    out = nc.dram_tensor("out", (T, D), BF16, kind="ExternalOutput")

    r = lambda ap: ap.bitcast(F32R)

    from contextlib import ExitStack
    with tile.TileContext(nc) as tc, ExitStack() as est:
        cp = est.enter_context(tc.tile_pool(name="consts", bufs=1))
        pp = est.enter_context(tc.tile_pool(name="persist", bufs=1))
        # PSUM pools: 1+2+1+2+2 = 8 banks
        paccp = est.enter_context(tc.tile_pool(name="pacc", bufs=1, space="PSUM"))
        stp = est.enter_context(tc.tile_pool(name="stp", bufs=2, space="PSUM"))
        pop = est.enter_context(tc.tile_pool(name="pop", bufs=1, space="PSUM"))
        auxp = est.enter_context(tc.tile_pool(name="auxp", bufs=2, space="PSUM"))
        outp = est.enter_context(tc.tile_pool(name="outp", bufs=2, space="PSUM"))
        # SBUF streaming pools
        hsp = est.enter_context(tc.tile_pool(name="hsp", bufs=2))
        pep = est.enter_context(tc.tile_pool(name="pep", bufs=2))
        accp = est.enter_context(tc.tile_pool(name="accp", bufs=2))
        nrm = est.enter_context(tc.tile_pool(name="nrm", bufs=2))
        ostp = est.enter_context(tc.tile_pool(name="ostp", bufs=2))

        # ---- small consts (emitted first: cheap DMAs, needed early) ----
        vkT_s = cp.tile([HD, R], F32R)
        vvT_s = cp.tile([HD, R], F32R)
        lkA_s = cp.tile([HD, RANK], F32R)
        lkB_s = cp.tile([RANK, HD], F32R)
        lvA_s = cp.tile([HD, RANK], F32R)
        lvB_s = cp.tile([RANK, HD], F32R)
        onesc_s = cp.tile([128, 1], F32R)
        onesr_s = cp.tile([1, 128], F32R)
        qw_s = cp.tile([HD, 1], F32)
        kw_s = cp.tile([HD, 1], F32)
        mtri_s = cp.tile([128, 128], F32)
        onesel_s = cp.tile([128, 10, 10], F32R)
        onesel2_s = cp.tile([10, 10, 128], F32R)
        rotm_s = cp.tile([HD, HD], F32R)
        ident_s = cp.tile([128, 128], F32R)
        epsc = cp.tile([128, 1], F32)
        nc.vector.memset(epsc[:], EPS)
        _deferred_const_dmas = lambda: None
        nc.sync.dma_start(vkT_s[:], r(vkT[:]))
        nc.sync.dma_start(vvT_s[:], r(vvT[:]))
        nc.sync.dma_start(lkA_s[:], r(lkA[:]))
        nc.sync.dma_start(lkB_s[:], r(lkB[:]))
        nc.sync.dma_start(lvA_s[:], r(lvA[:]))
        nc.sync.dma_start(lvB_s[:], r(lvB[:]))
        nc.sync.dma_start(onesc_s[:], r(onesc[:]))
        nc.sync.dma_start(onesr_s[:], r(onesr[:]))
        nc.sync.dma_start(qw_s[:], qw[:])
        nc.sync.dma_start(kw_s[:], kw[:])
        nc.sync.dma_start(mtri_s[:], mtri[:])
        nc.sync.dma_start(onesel_s[:], r(onesel[:]).rearrange("p (a b) -> p a b", a=10))
        nc.sync.dma_start(onesel2_s[:], r(onesel2[:]).rearrange("p (a b) -> p a b", a=10))
        nc.sync.dma_start(rotm_s[:], r(rotm[:]))
        nc.sync.dma_start(ident_s[:], r(ident[:]))

        # ---- big persistent tensors ----
        wq_s = pp.tile([128, ND, NQH * HD], BF16)
        wkv_s = pp.tile([128, ND, 2 * HD], BF16)
        wo_s = pp.tile([128, NQH, D], BF16)
        qT = [pp.tile([HD, T], F32, tag=f"qT{h}", name=f"qT{h}") for h in range(NQH)]
        kT = pp.tile([HD, R + T], F32)
        vnat = pp.tile([128, 128 + T], F32)   # cols 0:128 rows 0:64 = virtual V
        cosT_s = cp.tile([HD, T], F32)
        sinT_s = cp.tile([HD, T], F32)

        # hs chunk prefetch ring
        hs_tiles = [None] * NPC

        def hs_fetch(pc, split=False):
            t_ = hsp.tile([128, ND, TC], BF16, tag="hs")
            src = hsT[:, pc * TC:(pc + 1) * TC]
            if split:
                nc.sync.dma_start(
                    t_[:, 0:ND // 2, :],
                    src[0:D // 2, :].rearrange("(d p) t -> p d t", p=128))
                nc.sync.dma_start(
                    t_[:, ND // 2:ND, :],
                    src[D // 2:D, :].rearrange("(d p) t -> p d t", p=128))
            else:
                nc.sync.dma_start(t_[:], src.rearrange("(d p) t -> p d t", p=128))
            hs_tiles[pc] = t_

        # weight / hs DMA order = SP priority order: wkv first (k/v groups of
        # P0 can start earliest), hs0 in halves, then wq halves.
        nc.sync.dma_start(wkv_s[:], wkv[:].rearrange("(d p) c -> p d c", p=128))
        hs_fetch(0, split=True)
        nc.sync.dma_start(
            wq_s[:, :, 0:256], wq[:, 0:256].rearrange("(d p) c -> p d c", p=128))
        nc.sync.dma_start(
            wq_s[:, :, 256:512], wq[:, 256:512].rearrange("(d p) c -> p d c", p=128))
        hs_fetch(1)
        nc.sync.dma_start(cosT_s[:], cosT[:])
        nc.sync.dma_start(sinT_s[:], sinT[:])
        nc.sync.dma_start(wo_s[:], wo[:].rearrange("(h p) c -> p h c", p=128))

        # ================= LoRA-adapt virtual KV =================
        vvirtT = cp.tile([HD, R], F32)
        t1 = auxp.tile([128, 512], F32, tag="aux")
        nc.tensor.matmul(t1[0:RANK, 0:R], lkA_s[:], vkT_s[:], start=True, stop=True)
        t1s = cp.tile([RANK, R], F32R)
        nc.scalar.copy(t1s[:], t1[0:RANK, 0:R])
        t2 = auxp.tile([128, 512], F32, tag="aux")
        nc.tensor.matmul(t2[0:HD, 0:R], lkB_s[:], t1s[:], start=True, stop=True)
        nc.vector.tensor_add(kT[:, 0:R], vkT_s[:].bitcast(F32), t2[0:HD, 0:R])
        u1 = auxp.tile([128, 512], F32, tag="aux")
        nc.tensor.matmul(u1[0:RANK, 0:R], lvA_s[:], vvT_s[:], start=True, stop=True)
        u1s = cp.tile([RANK, R], F32R)
        nc.scalar.copy(u1s[:], u1[0:RANK, 0:R])
        u2 = auxp.tile([128, 512], F32, tag="aux")
        nc.tensor.matmul(u2[0:HD, 0:R], lvB_s[:], u1s[:], start=True, stop=True)
        nc.vector.tensor_add(vvirtT[:], vvT_s[:].bitcast(F32), u2[0:HD, 0:R])
        vtp = auxp.tile([128, 512], F32, tag="aux")
        nc.tensor.transpose(vtp[0:R, 0:HD].bitcast(F32R), r(vvirtT[:]), ident_s[:])
        nc.gpsimd.tensor_copy(vnat[0:R, 0:128], vtp[0:R, 0:HD])

        # ================= emission helpers =================
        def proj_chunk(pc):
            if pc + 2 < NPC:
                hs_fetch(pc + 2)
            hs_t = hs_tiles[pc]
            cs = pc * TC
            # k
            p = paccp.tile([128, TC], F32, tag="pacc")
            for d in range(ND):
                nc.tensor.matmul(p[:], wkv_s[:, d, 0:HD], hs_t[:, d, :],
                                 start=(d == 0), stop=(d == ND - 1))
            nc.gpsimd.tensor_copy(kT[:, R + cs:R + cs + TC], p[:])
            # v natural: two 128-token row blocks
            for vb in range(TC // 128):
                p = paccp.tile([128, TC], F32, tag="pacc")
                for d in range(ND):
                    nc.tensor.matmul(p[:, 0:HD], hs_t[:, d, vb * 128:(vb + 1) * 128],
                                     wkv_s[:, d, HD:2 * HD],
                                     start=(d == 0), stop=(d == ND - 1))
                bg = (cs + vb * 128) // 128
                nc.gpsimd.tensor_copy(vnat[:, (bg + 1) * 128:(bg + 2) * 128],
                                      p[:, 0:HD])
            # q heads
            for h in range(NQH):
                p = paccp.tile([128, TC], F32, tag="pacc")
                for d in range(ND):
                    nc.tensor.matmul(p[:], wq_s[:, d, h * HD:(h + 1) * HD],
                                     hs_t[:, d, :], start=(d == 0), stop=(d == ND - 1))
                nc.gpsimd.tensor_copy(qT[h][:, cs:cs + TC], p[:])

        def _targets(ncx):
            a, b = ncx * AC, (ncx + 1) * AC
            return [(qT[h][:, a:b], qw_s, a, b) for h in range(NQH)] + \
                   [(kT[:, R + a:R + b], kw_s, a, b)]

        def norm_half(c0, c1):
            # batched rsqrt: mean-square rows for all 10 (target,chunk) pairs
            # land in rows of one PSUM tile; one Sqrt + one reciprocal total.
            tgts = _targets(c0) + _targets(c1)
            msb = auxp.tile([128, AC], F32, tag="aux")
            for i, (xap, w, a, b) in enumerate(tgts):
                sq = nrm.tile([HD, AC], F32R, tag="sqt")
                nc.gpsimd.tensor_mul(sq[:].bitcast(F32), xap, xap)
                # selector column i: accumulates this pair's row-sum into row i
                nc.tensor.matmul(msb[0:10, :], onesel_s[:, i, :], sq[:],
                                 start=(i == 0), stop=(i == len(tgts) - 1))
            srt = nrm.tile([10, AC], F32, tag="srt", bufs=1)
            nc.scalar.activation(srt[:], msb[0:10, :], ACTF.Sqrt,
                                 bias=epsc[0:10, :], scale=1.0 / HD)
            rinv = nrm.tile([10, AC], F32R, tag="rinv", bufs=1)
            with nc.allow_low_precision(reason="f32r same width as f32"):
                nc.vector.reciprocal(rinv[:], srt[:])
            for i, (xap, w, a, b) in enumerate(tgts):
                nrb = auxp.tile([128, AC], F32, tag="aux")
                # row-selector broadcast: nrb[m,t] = rinv[i,t] for all m
                nc.tensor.matmul(nrb[:], onesel2_s[:, i, :], rinv[:],
                                 start=True, stop=True)
                xn = nrm.tile([HD, AC], F32R, tag="xn", bufs=1)
                nc.vector.scalar_tensor_tensor(xn[:], xap, w[:], nrb[:],
                                               op0=ALU.mult, op1=ALU.mult)
                pr = auxp.tile([128, AC], F32, tag="aux")
                nc.tensor.matmul(pr[:], rotm_s[:], xn[:], start=True, stop=True)
                # xn <- xn * cos (in place, after pr consumed xn)
                nc.gpsimd.tensor_mul(xn[:].bitcast(F32), xn[:].bitcast(F32),
                                     cosT_s[:, a:b])
                t2_ = nrm.tile([HD, AC], F32R, tag="sqt")
                nc.vector.tensor_mul(t2_[:].bitcast(F32), pr[:], sinT_s[:, a:b])
                nc.gpsimd.tensor_add(xap, xn[:].bitcast(F32), t2_[:].bitcast(F32))

        def att_head(tcj, h):
            cs = tcj * AC
            hq = qT[h]
            acc_t = accp.tile([128, AC], F32, tag="acc")
            po_t = pop.tile([128, AC], F32, tag="po")
            nreal = 4 * tcj + 4
            for b_ in range(nreal):
                diag = b_ >= 4 * tcj
                off = 128 * (b_ - 4 * tcj) if diag else 0
                n = AC - off
                st_t = stp.tile([128, AC], F32, tag="st")
                nc.tensor.matmul(st_t[:, off:AC],
                                 r(kT[:, R + b_ * 128:R + (b_ + 1) * 128]),
                                 r(hq[:, cs + off:cs + AC]), start=True, stop=True)
                if diag:
                    nc.vector.tensor_add(st_t[:, off:off + 128],
                                         st_t[:, off:off + 128], mtri_s[:])
                pe_t = pep.tile([128, AC], F32R, tag="pe")
                nc.scalar.activation(pe_t[:, off:AC], st_t[:, off:AC], ACTF.Exp,
                                     scale=SCALING)
                if b_ == 0:
                    nc.gpsimd.tensor_copy(acc_t[:], pe_t[:].bitcast(F32))
                else:
                    nc.vector.tensor_add(acc_t[:, off:AC], acc_t[:, off:AC],
                                         pe_t[:, off:AC].bitcast(F32))
                nc.tensor.matmul(po_t[:, off:AC],
                                 r(vnat[:, (b_ + 1) * 128:(b_ + 2) * 128]),
                                 pe_t[:, off:AC], start=(b_ == 0), stop=False)
            # virtual prefix block (full width, 64 rows)
            st_t = stp.tile([128, AC], F32, tag="st")
            nc.tensor.matmul(st_t[0:R, :], r(kT[:, 0:R]), r(hq[:, cs:cs + AC]),
                             start=True, stop=True)
            pe_t = pep.tile([128, AC], F32R, tag="pe")
            nc.scalar.activation(pe_t[0:R, :], st_t[0:R, :], ACTF.Exp, scale=SCALING)
            nc.vector.tensor_add(acc_t[0:R, :], acc_t[0:R, :],
                                 pe_t[0:R, :].bitcast(F32))
            nc.tensor.matmul(po_t[:], r(vnat[0:R, 0:128]), pe_t[0:R, :],
                             start=False, stop=True)
            # normalize: oT (bf16, aliased into qT storage) = po / den
            den = auxp.tile([128, AC], F32, tag="aux")
            nc.tensor.matmul(den[0:1, :], onesc_s[:], r(acc_t[:]),
                             start=True, stop=True)
            ari = nrm.tile([10, AC], F32R, tag="rinv", bufs=1)
            with nc.allow_low_precision(reason="f32r same width as f32"):
                nc.vector.reciprocal(ari[0:1, :], den[0:1, :])
            rb = auxp.tile([128, AC], F32, tag="aux")
            nc.tensor.matmul(rb[:], onesr_s[:], ari[0:1, :], start=True, stop=True)
            oTv = hq[:].bitcast(BF16)   # [128, 2*T] bf16; cols 0:T = oT
            nc.vector.tensor_mul(oTv[:, cs:cs + AC], po_t[:], rb[:])

        def out_chunk(c):
            for tt in range(4 * c, 4 * c + 4):
                for j2 in range(D // 512):
                    op = outp.tile([128, 512], F32, tag="opo")
                    for h in range(NQH):
                        oTv = qT[h][:].bitcast(BF16)
                        nc.tensor.matmul(op[:], oTv[:, tt * 128:(tt + 1) * 128],
                                         wo_s[:, h, j2 * 512:(j2 + 1) * 512],
                                         start=(h == 0), stop=(h == NQH - 1))
                    ost = ostp.tile([128, 512], BF16, tag="ost")
                    if (tt + j2) % 2 == 0:
                        nc.scalar.copy(ost[:], op[:])
                    else:
                        nc.vector.tensor_copy(ost[:], op[:])
                    nc.sync.dma_start(
                        out[tt * 128:(tt + 1) * 128, j2 * 512:(j2 + 1) * 512],
                        ost[:])

        # ================= master emission sequence =================
        proj_chunk(0)
        proj_chunk(1)
        proj_chunk(2)
        proj_chunk(3)
        norm_half(0, 1)
        for h in range(NQH):
            att_head(0, h)
        proj_chunk(4)
        proj_chunk(5)
        for h in range(NQH):
            att_head(1, h)
        out_chunk(0)
        proj_chunk(6)
        proj_chunk(7)
        norm_half(2, 3)
        for h in range(NQH):
            att_head(2, h)
        out_chunk(1)
        for h in range(NQH):
            att_head(3, h)
        out_chunk(2)
        out_chunk(3)

    nc.compile()
    return nc


_NC_CACHE = {}


def _get_nc():
    if "nc" not in _NC_CACHE:
        _NC_CACHE["nc"] = build_nc()
    return _NC_CACHE["nc"]


def kernel(**inputs) -> np.ndarray:
    f = lambda k: np.asarray(inputs[k], np.float32)
    bf = lambda a: np.ascontiguousarray(a).astype(ml_dtypes.bfloat16)
    hs = f("hidden_states")[0]            # (T, D)
    vk = f("virtual_keys")[0]             # (HKV, R, HD)
    vv = f("virtual_values")[0]
    Wq, Wk, Wv, Wo = f("Wq"), f("Wk"), f("Wv"), f("Wo")
    qnw, knw = f("q_norm_w"), f("k_norm_w")
    lkA, lkB = f("lora_k_A"), f("lora_k_B")
    lvA, lvB = f("lora_v_A"), f("lora_v_B")
    sk = np.float32(np.asarray(inputs["scale_k"]))
    sv = np.float32(np.asarray(inputs["scale_v"]))
    am = f("attention_mask")              # (1,1,T,T)
    cos, sin = f("cos"), f("sin")         # (T, HD)

    hsT = bf(hs.T)
    cosT = np.ascontiguousarray(cos.T)
    sinT = np.ascontiguousarray(sin.T)
    # aligned [128,128] causal triangle: rows k, cols q, masked iff k > q
    mtri = np.ascontiguousarray(am[0, 0, 0:128, 0:128].T)
    rotm = np.zeros((HD, HD), np.float32)
    for dd in range(64):
        rotm[dd + 64, dd] = -1.0          # rot[d] = -x[d+64], d<64
        rotm[dd, dd + 64] = 1.0           # rot[d] = +x[d-64], d>=64
    ident = np.eye(128, dtype=np.float32)
    onesc = np.ones((128, 1), np.float32)
    onesr = np.ones((1, 128), np.float32)
    onesel = np.zeros((128, 10, 10), np.float32)
    for i in range(10):
        onesel[:, i, i] = 1.0
    onesel = onesel.reshape(128, 100)
    onesel2 = np.zeros((10, 10, 128), np.float32)
    for i in range(10):
        onesel2[i, i, :] = 1.0
    onesel2 = onesel2.reshape(10, 1280)
    lkBs = np.ascontiguousarray(lkB * sk)
    lvBs = np.ascontiguousarray(lvB * sv)

    in_maps = []
    for m in range(8):
        in_maps.append({
            "hsT": hsT,
            "wq": bf(Wq[:, 512 * m:512 * (m + 1)]),
            "wkv": bf(np.concatenate(
                [Wk[:, 128 * m:128 * (m + 1)], Wv[:, 128 * m:128 * (m + 1)]],
                axis=1)),
            "wo": bf(Wo[512 * m:512 * (m + 1), :]),
            "vkT": np.ascontiguousarray(vk[m].T),
            "vvT": np.ascontiguousarray(vv[m].T),
            "lkA": lkA, "lkB": lkBs, "lvA": lvA, "lvB": lvBs,
            "qw": np.ascontiguousarray(qnw[:, None]),
            "kw": np.ascontiguousarray(knw[:, None]),
            "cosT": cosT, "sinT": sinT, "mtri": mtri,
            "rotm": rotm, "ident": ident, "onesc": onesc, "onesr": onesr,
            "onesel": onesel, "onesel2": onesel2,
        })

    nc = _get_nc()
    res = run_bass_kernel_spmd(nc, in_maps, core_ids=list(range(8)))
    acc = res.results[0]["out"].astype(np.float32)
    for m in range(1, 8):
        acc = acc + res.results[m]["out"].astype(np.float32)
    return acc[None]  # (1, T, D)


# revision 8
# speedup vs baseline: 1.3792x; 1.0686x over previous
"""Trainium2 Bass kernel for KVAdapterInjector (Qwen3-style GQA attention with
LoRA-adapted virtual KV prefix).

Sharding: tensor-parallel over heads across 8 cores. Core m gets KV head m and
Q heads 4m..4m+3. Wq/Wk/Wv sharded on output dim, Wo on input dim; partial
outputs summed on host.

v2 layout/scheduling notes:
- hs/Wq/Wk/Wv/Wo/out in bf16 (halves DMA; matmul rate identical to f32r).
- V is projected directly into natural [token, HD] layout (no transposes).
- Softmax denominators accumulated on DVE; rsqrt for RMSNorm computed as
  exp(-0.5*ln(x)) on Act so the whole kernel uses one activation table.
- Causal diagonal blocks are windowed (masked columns not computed); only a
  [128,128] triangle mask remains.
- Flat PSUM pools (2+2+2+2 banks) and interleaved emission so the tile
  scheduler overlaps projection / norm+rope / attention / out-projection.
"""
import sys

sys.path.insert(0, "/opt/trn_rl_repo")

import numpy as np
import ml_dtypes

import concourse.bass as bass
import concourse.mybir as mybir
import concourse.tile as tile
from concourse import bacc
from concourse.bass_utils import run_bass_kernel_spmd

F32 = mybir.dt.float32
F32R = mybir.dt.float32r
BF16 = mybir.dt.bfloat16
ALU = mybir.AluOpType
ACTF = mybir.ActivationFunctionType

T = 2048
D = 4096
HD = 128
NQH = 4          # q heads per core
R = 64           # virtual tokens
RANK = 16
EPS = 1e-6
SCALING = HD ** -0.5
ND = D // 128    # 32 contraction tiles
TC = 256         # projection T-chunk
NPC = T // TC    # 8 projection chunks
AC = 512         # attention / norm T-chunk
NAC = T // AC    # 4 attention chunks


def build_nc():
    nc = bacc.Bacc(None, target_bir_lowering=False, debug=False)

    # ---- DRAM I/O ----
    hsT = nc.dram_tensor("hsT", (D, T), BF16, kind="ExternalInput")
    wq = nc.dram_tensor("wq", (D, NQH * HD), BF16, kind="ExternalInput")
    wkv = nc.dram_tensor("wkv", (D, 2 * HD), BF16, kind="ExternalInput")
    wo = nc.dram_tensor("wo", (NQH * HD, D), BF16, kind="ExternalInput")
    vkT = nc.dram_tensor("vkT", (HD, R), F32, kind="ExternalInput")
    vvT = nc.dram_tensor("vvT", (HD, R), F32, kind="ExternalInput")
    lkA = nc.dram_tensor("lkA", (HD, RANK), F32, kind="ExternalInput")
    lkB = nc.dram_tensor("lkB", (RANK, HD), F32, kind="ExternalInput")  # pre-scaled
    lvA = nc.dram_tensor("lvA", (HD, RANK), F32, kind="ExternalInput")
    lvB = nc.dram_tensor("lvB", (RANK, HD), F32, kind="ExternalInput")  # pre-scaled
    qw = nc.dram_tensor("qw", (HD, 1), F32, kind="ExternalInput")
    kw = nc.dram_tensor("kw", (HD, 1), F32, kind="ExternalInput")
    cosT = nc.dram_tensor("cosT", (HD, T), F32, kind="ExternalInput")
    sinT = nc.dram_tensor("sinT", (HD, T), F32, kind="ExternalInput")
    mtri = nc.dram_tensor("mtri", (128, 128), F32, kind="ExternalInput")
    rotm = nc.dram_tensor("rotm", (HD, HD), F32, kind="ExternalInput")
    ident = nc.dram_tensor("ident", (128, 128), F32, kind="ExternalInput")
    onesc = nc.dram_tensor("onesc", (128, 1), F32, kind="ExternalInput")
    onesr = nc.dram_tensor("onesr", (1, 128), F32, kind="ExternalInput")
    onesel = nc.dram_tensor("onesel", (128, 100), F32, kind="ExternalInput")
    onesel2 = nc.dram_tensor("onesel2", (10, 1280), F32, kind="ExternalInput")
    out = nc.dram_tensor("out", (T, D), BF16, kind="ExternalOutput")

    r = lambda ap: ap.bitcast(F32R)

    from contextlib import ExitStack
    with tile.TileContext(nc) as tc, ExitStack() as est:
        cp = est.enter_context(tc.tile_pool(name="consts", bufs=1))
        pp = est.enter_context(tc.tile_pool(name="persist", bufs=1))
        # PSUM pools: 1+2+1+2+2 = 8 banks
        paccp = est.enter_context(tc.tile_pool(name="pacc", bufs=1, space="PSUM"))
        stp = est.enter_context(tc.tile_pool(name="stp", bufs=2, space="PSUM"))
        pop = est.enter_context(tc.tile_pool(name="pop", bufs=1, space="PSUM"))
        auxp = est.enter_context(tc.tile_pool(name="auxp", bufs=2, space="PSUM"))
        outp = est.enter_context(tc.tile_pool(name="outp", bufs=2, space="PSUM"))
        # SBUF streaming pools
        hsp = est.enter_context(tc.tile_pool(name="hsp", bufs=2))
        pep = est.enter_context(tc.tile_pool(name="pep", bufs=2))
        accp = est.enter_context(tc.tile_pool(name="accp", bufs=2))
        nrm = est.enter_context(tc.tile_pool(name="nrm", bufs=2))
        ostp = est.enter_context(tc.tile_pool(name="ostp", bufs=2))

        # ---- small consts (emitted first: cheap DMAs, needed early) ----
        vkT_s = cp.tile([HD, R], F32R)
        vvT_s = cp.tile([HD, R], F32R)
        lkA_s = cp.tile([HD, RANK], F32R)
        lkB_s = cp.tile([RANK, HD], F32R)
        lvA_s = cp.tile([HD, RANK], F32R)
        lvB_s = cp.tile([RANK, HD], F32R)
        onesc_s = cp.tile([128, 1], F32R)
        onesr_s = cp.tile([1, 128], F32R)
        qw_s = cp.tile([HD, 1], F32)
        kw_s = cp.tile([HD, 1], F32)
        mtri_s = cp.tile([128, 128], F32)
        onesel_s = cp.tile([128, 10, 10], F32R)
        onesel2_s = cp.tile([10, 10, 128], F32R)
        rotm_s = cp.tile([HD, HD], F32R)
        ident_s = cp.tile([128, 128], F32R)
        epsc = cp.tile([128, 1], F32)
        nc.vector.memset(epsc[:], EPS)

        def small_const_dmas():
            nc.sync.dma_start(vkT_s[:], r(vkT[:]))
            nc.sync.dma_start(vvT_s[:], r(vvT[:]))
            nc.sync.dma_start(lkA_s[:], r(lkA[:]))
            nc.sync.dma_start(lkB_s[:], r(lkB[:]))
            nc.sync.dma_start(lvA_s[:], r(lvA[:]))
            nc.sync.dma_start(lvB_s[:], r(lvB[:]))
            nc.sync.dma_start(onesc_s[:], r(onesc[:]))
            nc.sync.dma_start(onesr_s[:], r(onesr[:]))
            nc.sync.dma_start(qw_s[:], qw[:])
            nc.sync.dma_start(kw_s[:], kw[:])
            nc.sync.dma_start(mtri_s[:], mtri[:])
            nc.sync.dma_start(onesel_s[:],
                              r(onesel[:]).rearrange("p (a b) -> p a b", a=10))
            nc.sync.dma_start(onesel2_s[:],
                              r(onesel2[:]).rearrange("p (a b) -> p a b", a=10))
            nc.sync.dma_start(rotm_s[:], r(rotm[:]))
            nc.sync.dma_start(ident_s[:], r(ident[:]))

        # ---- big persistent tensors ----
        wq_s = pp.tile([128, ND, NQH * HD], BF16)
        wkv_s = pp.tile([128, ND, 2 * HD], BF16)
        wo_s = pp.tile([128, NQH, D], BF16)
        qT = [pp.tile([HD, T], F32, tag=f"qT{h}", name=f"qT{h}") for h in range(NQH)]
        kT = pp.tile([HD, R + T], F32)
        vnat = pp.tile([128, 128 + T], F32)   # cols 0:128 rows 0:64 = virtual V
        cosT_s = cp.tile([HD, T], F32)
        sinT_s = cp.tile([HD, T], F32)

        # hs chunk prefetch ring
        hs_tiles = [None] * NPC

        def hs_fetch(pc, eng=None):
            eng = eng or nc.sync
            t_ = hsp.tile([128, ND, TC], BF16, tag="hs")
            src = hsT[:, pc * TC:(pc + 1) * TC]
            eng.dma_start(
                t_[:, 0:ND // 2, :],
                src[0:D // 2, :].rearrange("(d p) t -> p d t", p=128))
            eng.dma_start(
                t_[:, ND // 2:ND, :],
                src[D // 2:D, :].rearrange("(d p) t -> p d t", p=128))
            hs_tiles[pc] = t_

        # startup: wkv+wq on the SP queue, hs0/hs1 on the Act HWDGE queue
        # (Act engine is idle at start) so P0 can begin ~6us in.
        nc.sync.dma_start(wkv_s[:], wkv[:].rearrange("(d p) c -> p d c", p=128))
        hs_fetch(0, eng=nc.scalar)
        nc.sync.dma_start(
            wq_s[:, :, 0:256], wq[:, 0:256].rearrange("(d p) c -> p d c", p=128))
        hs_fetch(1, eng=nc.scalar)
        nc.sync.dma_start(
            wq_s[:, :, 256:512], wq[:, 256:512].rearrange("(d p) c -> p d c", p=128))
        small_const_dmas()
        nc.sync.dma_start(cosT_s[:], cosT[:])
        nc.sync.dma_start(sinT_s[:], sinT[:])
        nc.sync.dma_start(wo_s[:], wo[:].rearrange("(h p) c -> p h c", p=128))

        # ================= LoRA-adapt virtual KV =================
        vvirtT = cp.tile([HD, R], F32)
        t1 = auxp.tile([128, 512], F32, tag="aux")
        nc.tensor.matmul(t1[0:RANK, 0:R], lkA_s[:], vkT_s[:], start=True, stop=True)
        t1s = cp.tile([RANK, R], F32R)
        nc.scalar.copy(t1s[:], t1[0:RANK, 0:R])
        t2 = auxp.tile([128, 512], F32, tag="aux")
        nc.tensor.matmul(t2[0:HD, 0:R], lkB_s[:], t1s[:], start=True, stop=True)
        nc.vector.tensor_add(kT[:, 0:R], vkT_s[:].bitcast(F32), t2[0:HD, 0:R])
        u1 = auxp.tile([128, 512], F32, tag="aux")
        nc.tensor.matmul(u1[0:RANK, 0:R], lvA_s[:], vvT_s[:], start=True, stop=True)
        u1s = cp.tile([RANK, R], F32R)
        nc.scalar.copy(u1s[:], u1[0:RANK, 0:R])
        u2 = auxp.tile([128, 512], F32, tag="aux")
        nc.tensor.matmul(u2[0:HD, 0:R], lvB_s[:], u1s[:], start=True, stop=True)
        nc.vector.tensor_add(vvirtT[:], vvT_s[:].bitcast(F32), u2[0:HD, 0:R])
        vtp = auxp.tile([128, 512], F32, tag="aux")
        nc.tensor.transpose(vtp[0:R, 0:HD].bitcast(F32R), r(vvirtT[:]), ident_s[:])
        nc.gpsimd.tensor_copy(vnat[0:R, 0:128], vtp[0:R, 0:HD])

        # ================= emission helpers =================
        def proj_chunk(pc):
            if pc + 2 < NPC:
                hs_fetch(pc + 2)
            hs_t = hs_tiles[pc]
            cs = pc * TC
            # k
            p = paccp.tile([128, TC], F32, tag="pacc")
            for d in range(ND):
                nc.tensor.matmul(p[:], wkv_s[:, d, 0:HD], hs_t[:, d, :],
                                 start=(d == 0), stop=(d == ND - 1))
            nc.gpsimd.tensor_copy(kT[:, R + cs:R + cs + TC], p[:])
            # v natural: two 128-token row blocks
            for vb in range(TC // 128):
                p = paccp.tile([128, TC], F32, tag="pacc")
                for d in range(ND):
                    nc.tensor.matmul(p[:, 0:HD], hs_t[:, d, vb * 128:(vb + 1) * 128],
                                     wkv_s[:, d, HD:2 * HD],
                                     start=(d == 0), stop=(d == ND - 1))
                bg = (cs + vb * 128) // 128
                nc.gpsimd.tensor_copy(vnat[:, (bg + 1) * 128:(bg + 2) * 128],
                                      p[:, 0:HD])
            # q heads
            for h in range(NQH):
                p = paccp.tile([128, TC], F32, tag="pacc")
                for d in range(ND):
                    nc.tensor.matmul(p[:], wq_s[:, d, h * HD:(h + 1) * HD],
                                     hs_t[:, d, :], start=(d == 0), stop=(d == ND - 1))
                nc.gpsimd.tensor_copy(qT[h][:, cs:cs + TC], p[:])

        def _targets(ncx):
            a, b = ncx * AC, (ncx + 1) * AC
            return [(qT[h][:, a:b], qw_s, a, b) for h in range(NQH)] + \
                   [(kT[:, R + a:R + b], kw_s, a, b)]

        def norm_half(c0, c1):
            # batched rsqrt: mean-square rows for all 10 (target,chunk) pairs
            # land in rows of one PSUM tile; one Sqrt + one reciprocal total.
            tgts = _targets(c0) + _targets(c1)
            msb = auxp.tile([128, AC], F32, tag="aux")
            for i, (xap, w, a, b) in enumerate(tgts):
                sq = nrm.tile([HD, AC], F32R, tag="sqt")
                nc.gpsimd.tensor_mul(sq[:].bitcast(F32), xap, xap)
                # selector column i: accumulates this pair's row-sum into row i
                nc.tensor.matmul(msb[0:10, :], onesel_s[:, i, :], sq[:],
                                 start=(i == 0), stop=(i == len(tgts) - 1))
            srt = nrm.tile([10, AC], F32, tag="srt", bufs=1)
            nc.scalar.activation(srt[:], msb[0:10, :], ACTF.Sqrt,
                                 bias=epsc[0:10, :], scale=1.0 / HD)
            rinv = nrm.tile([10, AC], F32R, tag="rinv", bufs=1)
            with nc.allow_low_precision(reason="f32r same width as f32"):
                nc.vector.reciprocal(rinv[:], srt[:])
            for i, (xap, w, a, b) in enumerate(tgts):
                nrb = auxp.tile([128, AC], F32, tag="aux")
                # row-selector broadcast: nrb[m,t] = rinv[i,t] for all m
                nc.tensor.matmul(nrb[:], onesel2_s[:, i, :], rinv[:],
                                 start=True, stop=True)
                xn = nrm.tile([HD, AC], F32R, tag="xn", bufs=1)
                nc.vector.scalar_tensor_tensor(xn[:], xap, w[:], nrb[:],
                                               op0=ALU.mult, op1=ALU.mult)
                pr = auxp.tile([128, AC], F32, tag="aux")
                nc.tensor.matmul(pr[:], rotm_s[:], xn[:], start=True, stop=True)
                # xn <- xn * cos (in place, after pr consumed xn)
                nc.gpsimd.tensor_mul(xn[:].bitcast(F32), xn[:].bitcast(F32),
                                     cosT_s[:, a:b])
                t2_ = nrm.tile([HD, AC], F32R, tag="sqt")
                nc.vector.tensor_mul(t2_[:].bitcast(F32), pr[:], sinT_s[:, a:b])
                nc.gpsimd.tensor_add(xap, xn[:].bitcast(F32), t2_[:].bitcast(F32))

        def att_head(tcj, h):
            cs = tcj * AC
            hq = qT[h]
            acc_t = accp.tile([128, AC], F32, tag="acc")
            po_t = pop.tile([128, AC], F32, tag="po")
            nreal = 4 * tcj + 4
            for b_ in range(nreal):
                diag = b_ >= 4 * tcj
                off = 128 * (b_ - 4 * tcj) if diag else 0
                n = AC - off
                st_t = stp.tile([128, AC], F32, tag="st")
                nc.tensor.matmul(st_t[:, off:AC],
                                 r(kT[:, R + b_ * 128:R + (b_ + 1) * 128]),
                                 r(hq[:, cs + off:cs + AC]), start=True, stop=True)
                if diag:
                    nc.vector.tensor_add(st_t[:, off:off + 128],
                                         st_t[:, off:off + 128], mtri_s[:])
                pe_t = pep.tile([128, AC], F32R, tag="pe")
                nc.scalar.activation(pe_t[:, off:AC], st_t[:, off:AC], ACTF.Exp,
                                     scale=SCALING)
                if b_ == 0:
                    nc.gpsimd.tensor_copy(acc_t[:], pe_t[:].bitcast(F32))
                else:
                    nc.vector.tensor_add(acc_t[:, off:AC], acc_t[:, off:AC],
                                         pe_t[:, off:AC].bitcast(F32))
                nc.tensor.matmul(po_t[:, off:AC],
                                 r(vnat[:, (b_ + 1) * 128:(b_ + 2) * 128]),
                                 pe_t[:, off:AC], start=(b_ == 0), stop=False)
            # virtual prefix block (full width, 64 rows)
            st_t = stp.tile([128, AC], F32, tag="st")
            nc.tensor.matmul(st_t[0:R, :], r(kT[:, 0:R]), r(hq[:, cs:cs + AC]),
                             start=True, stop=True)
            pe_t = pep.tile([128, AC], F32R, tag="pe")
            nc.scalar.activation(pe_t[0:R, :], st_t[0:R, :], ACTF.Exp, scale=SCALING)
            nc.vector.tensor_add(acc_t[0:R, :], acc_t[0:R, :],
                                 pe_t[0:R, :].bitcast(F32))
            nc.tensor.matmul(po_t[:], r(vnat[0:R, 0:128]), pe_t[0:R, :],
                             start=False, stop=True)
            # normalize: oT (bf16, aliased into qT storage) = po / den
            den = auxp.tile([128, AC], F32, tag="aux")
            nc.tensor.matmul(den[0:1, :], onesc_s[:], r(acc_t[:]),
                             start=True, stop=True)
            ari = nrm.tile([10, AC], F32R, tag="rinv", bufs=1)
            with nc.allow_low_precision(reason="f32r same width as f32"):
                nc.vector.reciprocal(ari[0:1, :], den[0:1, :])
            rb = auxp.tile([128, AC], F32, tag="aux")
            nc.tensor.matmul(rb[:], onesr_s[:], ari[0:1, :], start=True, stop=True)
            oTv = hq[:].bitcast(BF16)   # [128, 2*T] bf16; cols 0:T = oT
            nc.vector.tensor_mul(oTv[:, cs:cs + AC], po_t[:], rb[:])

        def out_chunk(c):
            for tt in range(4 * c, 4 * c + 4):
                for j2 in range(D // 512):
                    op = outp.tile([128, 512], F32, tag="opo")
                    for h in range(NQH):
                        oTv = qT[h][:].bitcast(BF16)
                        nc.tensor.matmul(op[:], oTv[:, tt * 128:(tt + 1) * 128],
                                         wo_s[:, h, j2 * 512:(j2 + 1) * 512],
                                         start=(h == 0), stop=(h == NQH - 1))
                    ost = ostp.tile([128, 512], BF16, tag="ost")
                    nc.gpsimd.tensor_copy(ost[:], op[:])
                    nc.sync.dma_start(
                        out[tt * 128:(tt + 1) * 128, j2 * 512:(j2 + 1) * 512],
                        ost[:])

        # ================= master emission sequence =================
        proj_chunk(0)
        proj_chunk(1)
        proj_chunk(2)
        proj_chunk(3)
        norm_half(0, 1)
        for h in range(NQH):
            att_head(0, h)
        proj_chunk(4)
        proj_chunk(5)
        for h in range(NQH):
            att_head(1, h)
        out_chunk(0)
        proj_chunk(6)
        proj_chunk(7)
        norm_half(2, 3)
        for h in range(NQH):
            att_head(2, h)
        out_chunk(1)
        for h in range(NQH):
            att_head(3, h)
        out_chunk(2)
        out_chunk(3)

    nc.compile()
    return nc


_NC_CACHE = {}


def _get_nc():
    if "nc" not in _NC_CACHE:
        _NC_CACHE["nc"] = build_nc()
    return _NC_CACHE["nc"]


def kernel(**inputs) -> np.ndarray:
    f = lambda k: np.asarray(inputs[k], np.float32)
    bf = lambda a: np.ascontiguousarray(a).astype(ml_dtypes.bfloat16)
    hs = f("hidden_states")[0]            # (T, D)
    vk = f("virtual_keys")[0]             # (HKV, R, HD)
    vv = f("virtual_values")[0]
    Wq, Wk, Wv, Wo = f("Wq"), f("Wk"), f("Wv"), f("Wo")
    qnw, knw = f("q_norm_w"), f("k_norm_w")
    lkA, lkB = f("lora_k_A"), f("lora_k_B")
    lvA, lvB = f("lora_v_A"), f("lora_v_B")
    sk = np.float32(np.asarray(inputs["scale_k"]))
    sv = np.float32(np.asarray(inputs["scale_v"]))
    am = f("attention_mask")              # (1,1,T,T)
    cos, sin = f("cos"), f("sin")         # (T, HD)

    hsT = bf(hs.T)
    cosT = np.ascontiguousarray(cos.T)
    sinT = np.ascontiguousarray(sin.T)
    # aligned [128,128] causal triangle: rows k, cols q, masked iff k > q
    mtri = np.ascontiguousarray(am[0, 0, 0:128, 0:128].T)
    rotm = np.zeros((HD, HD), np.float32)
    for dd in range(64):
        rotm[dd + 64, dd] = -1.0          # rot[d] = -x[d+64], d<64
        rotm[dd, dd + 64] = 1.0           # rot[d] = +x[d-64], d>=64
    ident = np.eye(128, dtype=np.float32)
    onesc = np.ones((128, 1), np.float32)
    onesr = np.ones((1, 128), np.float32)
    onesel = np.zeros((128, 10, 10), np.float32)
    for i in range(10):
        onesel[:, i, i] = 1.0
    onesel = onesel.reshape(128, 100)
    onesel2 = np.zeros((10, 10, 128), np.float32)
    for i in range(10):
        onesel2[i, i, :] = 1.0
    onesel2 = onesel2.reshape(10, 1280)
    lkBs = np.ascontiguousarray(lkB * sk)
    lvBs = np.ascontiguousarray(lvB * sv)

    in_maps = []
    for m in range(8):
        in_maps.append({
            "hsT": hsT,
            "wq": bf(Wq[:, 512 * m:512 * (m + 1)]),
            "wkv": bf(np.concatenate(
                [Wk[:, 128 * m:128 * (m + 1)], Wv[:, 128 * m:128 * (m + 1)]],
                axis=1)),
            "wo": bf(Wo[512 * m:512 * (m + 1), :]),
            "vkT": np.ascontiguousarray(vk[m].T),
            "vvT": np.ascontiguousarray(vv[m].T),
            "lkA": lkA, "lkB": lkBs, "lvA": lvA, "lvB": lvBs,
            "qw": np.ascontiguousarray(qnw[:, None]),
            "kw": np.ascontiguousarray(knw[:, None]),
            "cosT": cosT, "sinT": sinT, "mtri": mtri,
            "rotm": rotm, "ident": ident, "onesc": onesc, "onesr": onesr,
            "onesel": onesel, "onesel2": onesel2,
        })

    nc = _get_nc()
    res = run_bass_kernel_spmd(nc, in_maps, core_ids=list(range(8)))
    acc = res.results[0]["out"].astype(np.float32)
    for m in range(1, 8):
        acc = acc + res.results[m]["out"].astype(np.float32)
    return acc[None]  # (1, T, D)


# revision 9
# speedup vs baseline: 1.3833x; 1.0030x over previous
"""Trainium2 Bass kernel for KVAdapterInjector (Qwen3-style GQA attention with
LoRA-adapted virtual KV prefix).

Sharding: tensor-parallel over heads across 8 cores. Core m gets KV head m and
Q heads 4m..4m+3. Wq/Wk/Wv sharded on output dim, Wo on input dim; partial
outputs summed on host.

v2 layout/scheduling notes:
- hs/Wq/Wk/Wv/Wo/out in bf16 (halves DMA; matmul rate identical to f32r).
- V is projected directly into natural [token, HD] layout (no transposes).
- Softmax denominators accumulated on DVE; rsqrt for RMSNorm computed as
  exp(-0.5*ln(x)) on Act so the whole kernel uses one activation table.
- Causal diagonal blocks are windowed (masked columns not computed); only a
  [128,128] triangle mask remains.
- Flat PSUM pools (2+2+2+2 banks) and interleaved emission so the tile
  scheduler overlaps projection / norm+rope / attention / out-projection.
"""
import sys

sys.path.insert(0, "/opt/trn_rl_repo")

import numpy as np
import ml_dtypes

import concourse.bass as bass
import concourse.mybir as mybir
import concourse.tile as tile
from concourse import bacc
from concourse.bass_utils import run_bass_kernel_spmd

F32 = mybir.dt.float32
F32R = mybir.dt.float32r
BF16 = mybir.dt.bfloat16
ALU = mybir.AluOpType
ACTF = mybir.ActivationFunctionType

T = 2048
D = 4096
HD = 128
NQH = 4          # q heads per core
R = 64           # virtual tokens
RANK = 16
EPS = 1e-6
SCALING = HD ** -0.5
ND = D // 128    # 32 contraction tiles
TC = 256         # projection T-chunk
NPC = T // TC    # 8 projection chunks
AC = 512         # attention / norm T-chunk
NAC = T // AC    # 4 attention chunks


def build_nc():
    nc = bacc.Bacc(None, target_bir_lowering=False, debug=False)

    # ---- DRAM I/O ----
    hsT = nc.dram_tensor("hsT", (D, T), BF16, kind="ExternalInput")
    wq = nc.dram_tensor("wq", (D, NQH * HD), BF16, kind="ExternalInput")
    wkv = nc.dram_tensor("wkv", (D, 2 * HD), BF16, kind="ExternalInput")
    wo = nc.dram_tensor("wo", (NQH * HD, D), BF16, kind="ExternalInput")
    vkT = nc.dram_tensor("vkT", (HD, R), F32, kind="ExternalInput")
    vvT = nc.dram_tensor("vvT", (HD, R), F32, kind="ExternalInput")
    lkA = nc.dram_tensor("lkA", (HD, RANK), F32, kind="ExternalInput")
    lkB = nc.dram_tensor("lkB", (RANK, HD), F32, kind="ExternalInput")  # pre-scaled
    lvA = nc.dram_tensor("lvA", (HD, RANK), F32, kind="ExternalInput")
    lvB = nc.dram_tensor("lvB", (RANK, HD), F32, kind="ExternalInput")  # pre-scaled
    qw = nc.dram_tensor("qw", (HD, 1), F32, kind="ExternalInput")
    kw = nc.dram_tensor("kw", (HD, 1), F32, kind="ExternalInput")
    cosT = nc.dram_tensor("cosT", (HD, T), F32, kind="ExternalInput")
    sinT = nc.dram_tensor("sinT", (HD, T), F32, kind="ExternalInput")
    mtri = nc.dram_tensor("mtri", (128, 128), F32, kind="ExternalInput")
    rotm = nc.dram_tensor("rotm", (HD, HD), F32, kind="ExternalInput")
    ident = nc.dram_tensor("ident", (128, 128), F32, kind="ExternalInput")
    onesc = nc.dram_tensor("onesc", (128, 1), F32, kind="ExternalInput")
    onesr = nc.dram_tensor("onesr", (1, 128), F32, kind="ExternalInput")
    onesel = nc.dram_tensor("onesel", (128, 100), F32, kind="ExternalInput")
    onesel2 = nc.dram_tensor("onesel2", (10, 1280), F32, kind="ExternalInput")
    out = nc.dram_tensor("out", (T, D), BF16, kind="ExternalOutput")

    r = lambda ap: ap.bitcast(F32R)

    from contextlib import ExitStack
    with tile.TileContext(nc) as tc, ExitStack() as est:
        cp = est.enter_context(tc.tile_pool(name="consts", bufs=1))
        pp = est.enter_context(tc.tile_pool(name="persist", bufs=1))
        # PSUM pools: 1+2+1+2+2 = 8 banks
        paccp = est.enter_context(tc.tile_pool(name="pacc", bufs=1, space="PSUM"))
        stp = est.enter_context(tc.tile_pool(name="stp", bufs=2, space="PSUM"))
        pop = est.enter_context(tc.tile_pool(name="pop", bufs=1, space="PSUM"))
        auxp = est.enter_context(tc.tile_pool(name="auxp", bufs=2, space="PSUM"))
        outp = est.enter_context(tc.tile_pool(name="outp", bufs=2, space="PSUM"))
        # SBUF streaming pools
        hsp = est.enter_context(tc.tile_pool(name="hsp", bufs=2))
        pep = est.enter_context(tc.tile_pool(name="pep", bufs=2))
        accp = est.enter_context(tc.tile_pool(name="accp", bufs=2))
        nrm = est.enter_context(tc.tile_pool(name="nrm", bufs=2))
        ostp = est.enter_context(tc.tile_pool(name="ostp", bufs=3))

        # ---- small consts (emitted first: cheap DMAs, needed early) ----
        vkT_s = cp.tile([HD, R], F32R)
        vvT_s = cp.tile([HD, R], F32R)
        lkA_s = cp.tile([HD, RANK], F32R)
        lkB_s = cp.tile([RANK, HD], F32R)
        lvA_s = cp.tile([HD, RANK], F32R)
        lvB_s = cp.tile([RANK, HD], F32R)
        onesc_s = cp.tile([128, 1], F32R)
        onesr_s = cp.tile([1, 128], F32R)
        qw_s = cp.tile([HD, 1], F32)
        kw_s = cp.tile([HD, 1], F32)
        mtri_s = cp.tile([128, 128], F32)
        onesel_s = cp.tile([128, 10, 10], F32R)
        onesel2_s = cp.tile([10, 10, 128], F32R)
        rotm_s = cp.tile([HD, HD], F32R)
        ident_s = cp.tile([128, 128], F32R)
        epsc = cp.tile([128, 1], F32)
        nc.vector.memset(epsc[:], EPS)

        def small_const_dmas():
            nc.sync.dma_start(vkT_s[:], r(vkT[:]))
            nc.sync.dma_start(vvT_s[:], r(vvT[:]))
            nc.sync.dma_start(lkA_s[:], r(lkA[:]))
            nc.sync.dma_start(lkB_s[:], r(lkB[:]))
            nc.sync.dma_start(lvA_s[:], r(lvA[:]))
            nc.sync.dma_start(lvB_s[:], r(lvB[:]))
            nc.sync.dma_start(onesc_s[:], r(onesc[:]))
            nc.sync.dma_start(onesr_s[:], r(onesr[:]))
            nc.sync.dma_start(qw_s[:], qw[:])
            nc.sync.dma_start(kw_s[:], kw[:])
            nc.sync.dma_start(mtri_s[:], mtri[:])
            nc.sync.dma_start(onesel_s[:],
                              r(onesel[:]).rearrange("p (a b) -> p a b", a=10))
            nc.sync.dma_start(onesel2_s[:],
                              r(onesel2[:]).rearrange("p (a b) -> p a b", a=10))
            nc.sync.dma_start(rotm_s[:], r(rotm[:]))
            nc.sync.dma_start(ident_s[:], r(ident[:]))

        # ---- big persistent tensors ----
        wq_s = pp.tile([128, ND, NQH * HD], BF16)
        wkv_s = pp.tile([128, ND, 2 * HD], BF16)
        wo_s = pp.tile([128, NQH, D], BF16)
        qT = [pp.tile([HD, T], F32, tag=f"qT{h}", name=f"qT{h}") for h in range(NQH)]
        kT = pp.tile([HD, R + T], F32)
        vnat = pp.tile([128, 128 + T], F32)   # cols 0:128 rows 0:64 = virtual V
        cosT_s = cp.tile([HD, T], F32)
        sinT_s = cp.tile([HD, T], F32)

        # hs chunk prefetch ring
        hs_tiles = [None] * NPC

        def hs_fetch(pc, eng=None):
            eng = eng or nc.sync
            t_ = hsp.tile([128, ND, TC], BF16, tag="hs")
            src = hsT[:, pc * TC:(pc + 1) * TC]
            eng.dma_start(
                t_[:, 0:ND // 2, :],
                src[0:D // 2, :].rearrange("(d p) t -> p d t", p=128))
            eng.dma_start(
                t_[:, ND // 2:ND, :],
                src[D // 2:D, :].rearrange("(d p) t -> p d t", p=128))
            hs_tiles[pc] = t_

        # startup: wkv+wq on the SP queue, hs0/hs1 on the Act HWDGE queue
        # (Act engine is idle at start) so P0 can begin ~6us in.
        nc.sync.dma_start(wkv_s[:], wkv[:].rearrange("(d p) c -> p d c", p=128))
        hs_fetch(0, eng=nc.scalar)
        nc.sync.dma_start(
            wq_s[:, :, 0:256], wq[:, 0:256].rearrange("(d p) c -> p d c", p=128))
        hs_fetch(1, eng=nc.scalar)
        nc.sync.dma_start(
            wq_s[:, :, 256:512], wq[:, 256:512].rearrange("(d p) c -> p d c", p=128))
        small_const_dmas()
        nc.sync.dma_start(cosT_s[:], cosT[:])
        nc.sync.dma_start(sinT_s[:], sinT[:])
        nc.sync.dma_start(wo_s[:], wo[:].rearrange("(h p) c -> p h c", p=128))

        # ================= LoRA-adapt virtual KV =================
        vvirtT = cp.tile([HD, R], F32)
        t1 = auxp.tile([128, 512], F32, tag="aux")
        nc.tensor.matmul(t1[0:RANK, 0:R], lkA_s[:], vkT_s[:], start=True, stop=True)
        t1s = cp.tile([RANK, R], F32R)
        nc.scalar.copy(t1s[:], t1[0:RANK, 0:R])
        t2 = auxp.tile([128, 512], F32, tag="aux")
        nc.tensor.matmul(t2[0:HD, 0:R], lkB_s[:], t1s[:], start=True, stop=True)
        nc.vector.tensor_add(kT[:, 0:R], vkT_s[:].bitcast(F32), t2[0:HD, 0:R])
        u1 = auxp.tile([128, 512], F32, tag="aux")
        nc.tensor.matmul(u1[0:RANK, 0:R], lvA_s[:], vvT_s[:], start=True, stop=True)
        u1s = cp.tile([RANK, R], F32R)
        nc.scalar.copy(u1s[:], u1[0:RANK, 0:R])
        u2 = auxp.tile([128, 512], F32, tag="aux")
        nc.tensor.matmul(u2[0:HD, 0:R], lvB_s[:], u1s[:], start=True, stop=True)
        nc.vector.tensor_add(vvirtT[:], vvT_s[:].bitcast(F32), u2[0:HD, 0:R])
        vtp = auxp.tile([128, 512], F32, tag="aux")
        nc.tensor.transpose(vtp[0:R, 0:HD].bitcast(F32R), r(vvirtT[:]), ident_s[:])
        nc.gpsimd.tensor_copy(vnat[0:R, 0:128], vtp[0:R, 0:HD])

        # ================= emission helpers =================
        def proj_chunk(pc):
            if pc + 2 < NPC:
                hs_fetch(pc + 2)
            hs_t = hs_tiles[pc]
            cs = pc * TC
            # k
            p = paccp.tile([128, TC], F32, tag="pacc")
            for d in range(ND):
                nc.tensor.matmul(p[:], wkv_s[:, d, 0:HD], hs_t[:, d, :],
                                 start=(d == 0), stop=(d == ND - 1))
            nc.gpsimd.tensor_copy(kT[:, R + cs:R + cs + TC], p[:])
            # v natural: two 128-token row blocks
            for vb in range(TC // 128):
                p = paccp.tile([128, TC], F32, tag="pacc")
                for d in range(ND):
                    nc.tensor.matmul(p[:, 0:HD], hs_t[:, d, vb * 128:(vb + 1) * 128],
                                     wkv_s[:, d, HD:2 * HD],
                                     start=(d == 0), stop=(d == ND - 1))
                bg = (cs + vb * 128) // 128
                nc.gpsimd.tensor_copy(vnat[:, (bg + 1) * 128:(bg + 2) * 128],
                                      p[:, 0:HD])
            # q heads
            for h in range(NQH):
                p = paccp.tile([128, TC], F32, tag="pacc")
                for d in range(ND):
                    nc.tensor.matmul(p[:], wq_s[:, d, h * HD:(h + 1) * HD],
                                     hs_t[:, d, :], start=(d == 0), stop=(d == ND - 1))
                nc.gpsimd.tensor_copy(qT[h][:, cs:cs + TC], p[:])

        def _targets(ncx):
            a, b = ncx * AC, (ncx + 1) * AC
            return [(qT[h][:, a:b], qw_s, a, b) for h in range(NQH)] + \
                   [(kT[:, R + a:R + b], kw_s, a, b)]

        def norm_half(c0, c1):
            # batched rsqrt: mean-square rows for all 10 (target,chunk) pairs
            # land in rows of one PSUM tile; one Sqrt + one reciprocal total.
            tgts = _targets(c0) + _targets(c1)
            msb = auxp.tile([128, AC], F32, tag="aux")
            for i, (xap, w, a, b) in enumerate(tgts):
                sq = nrm.tile([HD, AC], F32R, tag="sqt")
                nc.gpsimd.tensor_mul(sq[:].bitcast(F32), xap, xap)
                # selector column i: accumulates this pair's row-sum into row i
                nc.tensor.matmul(msb[0:10, :], onesel_s[:, i, :], sq[:],
                                 start=(i == 0), stop=(i == len(tgts) - 1))
            srt = nrm.tile([10, AC], F32, tag="srt", bufs=1)
            nc.scalar.activation(srt[:], msb[0:10, :], ACTF.Sqrt,
                                 bias=epsc[0:10, :], scale=1.0 / HD)
            rinv = nrm.tile([10, AC], F32R, tag="rinv", bufs=1)
            with nc.allow_low_precision(reason="f32r same width as f32"):
                nc.vector.reciprocal(rinv[:], srt[:])
            for i, (xap, w, a, b) in enumerate(tgts):
                nrb = auxp.tile([128, AC], F32, tag="aux")
                # row-selector broadcast: nrb[m,t] = rinv[i,t] for all m
                nc.tensor.matmul(nrb[:], onesel2_s[:, i, :], rinv[:],
                                 start=True, stop=True)
                xn = nrm.tile([HD, AC], F32R, tag="xn", bufs=1)
                nc.vector.scalar_tensor_tensor(xn[:], xap, w[:], nrb[:],
                                               op0=ALU.mult, op1=ALU.mult)
                pr = auxp.tile([128, AC], F32, tag="aux")
                nc.tensor.matmul(pr[:], rotm_s[:], xn[:], start=True, stop=True)
                # xn <- xn * cos (in place, after pr consumed xn)
                nc.gpsimd.tensor_mul(xn[:].bitcast(F32), xn[:].bitcast(F32),
                                     cosT_s[:, a:b])
                t2_ = nrm.tile([HD, AC], F32R, tag="sqt")
                nc.vector.tensor_mul(t2_[:].bitcast(F32), pr[:], sinT_s[:, a:b])
                nc.gpsimd.tensor_add(xap, xn[:].bitcast(F32), t2_[:].bitcast(F32))

        def att_head(tcj, h):
            cs = tcj * AC
            hq = qT[h]
            acc_t = accp.tile([128, AC], F32, tag="acc")
            po_t = pop.tile([128, AC], F32, tag="po")
            nreal = 4 * tcj + 4
            for b_ in range(nreal):
                diag = b_ >= 4 * tcj
                off = 128 * (b_ - 4 * tcj) if diag else 0
                n = AC - off
                st_t = stp.tile([128, AC], F32, tag="st")
                nc.tensor.matmul(st_t[:, off:AC],
                                 r(kT[:, R + b_ * 128:R + (b_ + 1) * 128]),
                                 r(hq[:, cs + off:cs + AC]), start=True, stop=True)
                if diag:
                    nc.vector.tensor_add(st_t[:, off:off + 128],
                                         st_t[:, off:off + 128], mtri_s[:])
                pe_t = pep.tile([128, AC], F32R, tag="pe")
                nc.scalar.activation(pe_t[:, off:AC], st_t[:, off:AC], ACTF.Exp,
                                     scale=SCALING)
                if b_ == 0:
                    nc.gpsimd.tensor_copy(acc_t[:], pe_t[:].bitcast(F32))
                else:
                    nc.vector.tensor_add(acc_t[:, off:AC], acc_t[:, off:AC],
                                         pe_t[:, off:AC].bitcast(F32))
                nc.tensor.matmul(po_t[:, off:AC],
                                 r(vnat[:, (b_ + 1) * 128:(b_ + 2) * 128]),
                                 pe_t[:, off:AC], start=(b_ == 0), stop=False)
            # virtual prefix block (full width, 64 rows)
            st_t = stp.tile([128, AC], F32, tag="st")
            nc.tensor.matmul(st_t[0:R, :], r(kT[:, 0:R]), r(hq[:, cs:cs + AC]),
                             start=True, stop=True)
            pe_t = pep.tile([128, AC], F32R, tag="pe")
            nc.scalar.activation(pe_t[0:R, :], st_t[0:R, :], ACTF.Exp, scale=SCALING)
            nc.vector.tensor_add(acc_t[0:R, :], acc_t[0:R, :],
                                 pe_t[0:R, :].bitcast(F32))
            nc.tensor.matmul(po_t[:], r(vnat[0:R, 0:128]), pe_t[0:R, :],
                             start=False, stop=True)
            # normalize: oT (bf16, aliased into qT storage) = po / den
            den = auxp.tile([128, AC], F32, tag="aux")
            nc.tensor.matmul(den[0:1, :], onesc_s[:], r(acc_t[:]),
                             start=True, stop=True)
            ari = nrm.tile([10, AC], F32R, tag="rinv", bufs=1)
            with nc.allow_low_precision(reason="f32r same width as f32"):
                nc.vector.reciprocal(ari[0:1, :], den[0:1, :])
            rb = auxp.tile([128, AC], F32, tag="aux")
            nc.tensor.matmul(rb[:], onesr_s[:], ari[0:1, :], start=True, stop=True)
            oTv = hq[:].bitcast(BF16)   # [128, 2*T] bf16; cols 0:T = oT
            nc.vector.tensor_mul(oTv[:, cs:cs + AC], po_t[:], rb[:])

        def out_chunk(c):
            for tt in range(4 * c, 4 * c + 4):
                for j2 in range(D // 512):
                    op = outp.tile([128, 512], F32, tag="opo")
                    for h in range(NQH):
                        oTv = qT[h][:].bitcast(BF16)
                        nc.tensor.matmul(op[:], oTv[:, tt * 128:(tt + 1) * 128],
                                         wo_s[:, h, j2 * 512:(j2 + 1) * 512],
                                         start=(h == 0), stop=(h == NQH - 1))
                    ost = ostp.tile([128, 512], BF16, tag="ost")
                    eng = (nc.gpsimd, nc.vector, nc.scalar)[(4 * tt + j2) % 3]
                    if eng is nc.scalar:
                        nc.scalar.copy(ost[:], op[:])
                    else:
                        eng.tensor_copy(ost[:], op[:])
                    nc.sync.dma_start(
                        out[tt * 128:(tt + 1) * 128, j2 * 512:(j2 + 1) * 512],
                        ost[:])

        # ================= master emission sequence =================
        proj_chunk(0)
        proj_chunk(1)
        proj_chunk(2)
        proj_chunk(3)
        norm_half(0, 1)
        for h in range(NQH):
            att_head(0, h)
        proj_chunk(4)
        proj_chunk(5)
        for h in range(NQH):
            att_head(1, h)
        out_chunk(0)
        proj_chunk(6)
        proj_chunk(7)
        norm_half(2, 3)
        for h in range(NQH):
            att_head(2, h)
        out_chunk(1)
        for h in range(NQH):
            att_head(3, h)
        out_chunk(2)
        out_chunk(3)

    nc.compile()
    return nc


_NC_CACHE = {}


def _get_nc():
    if "nc" not in _NC_CACHE:
        _NC_CACHE["nc"] = build_nc()
    return _NC_CACHE["nc"]


def kernel(**inputs) -> np.ndarray:
    f = lambda k: np.asarray(inputs[k], np.float32)
    bf = lambda a: np.ascontiguousarray(a).astype(ml_dtypes.bfloat16)
    hs = f("hidden_states")[0]            # (T, D)
    vk = f("virtual_keys")[0]             # (HKV, R, HD)
    vv = f("virtual_values")[0]
    Wq, Wk, Wv, Wo = f("Wq"), f("Wk"), f("Wv"), f("Wo")
    qnw, knw = f("q_norm_w"), f("k_norm_w")
    lkA, lkB = f("lora_k_A"), f("lora_k_B")
    lvA, lvB = f("lora_v_A"), f("lora_v_B")
    sk = np.float32(np.asarray(inputs["scale_k"]))
    sv = np.float32(np.asarray(inputs["scale_v"]))
    am = f("attention_mask")              # (1,1,T,T)
    cos, sin = f("cos"), f("sin")         # (T, HD)

    hsT = bf(hs.T)
    cosT = np.ascontiguousarray(cos.T)
    sinT = np.ascontiguousarray(sin.T)
    # aligned [128,128] causal triangle: rows k, cols q, masked iff k > q
    mtri = np.ascontiguousarray(am[0, 0, 0:128, 0:128].T)
    rotm = np.zeros((HD, HD), np.float32)
    for dd in range(64):
        rotm[dd + 64, dd] = -1.0          # rot[d] = -x[d+64], d<64
        rotm[dd, dd + 64] = 1.0           # rot[d] = +x[d-64], d>=64
    ident = np.eye(128, dtype=np.float32)
    onesc = np.ones((128, 1), np.float32)
    onesr = np.ones((1, 128), np.float32)
    onesel = np.zeros((128, 10, 10), np.float32)
    for i in range(10):
        onesel[:, i, i] = 1.0
    onesel = onesel.reshape(128, 100)
    onesel2 = np.zeros((10, 10, 128), np.float32)
    for i in range(10):
        onesel2[i, i, :] = 1.0
    onesel2 = onesel2.reshape(10, 1280)
    lkBs = np.ascontiguousarray(lkB * sk)
    lvBs = np.ascontiguousarray(lvB * sv)

    in_maps = []
    for m in range(8):
        in_maps.append({
            "hsT": hsT,
            "wq": bf(Wq[:, 512 * m:512 * (m + 1)]),
            "wkv": bf(np.concatenate(
                [Wk[:, 128 * m:128 * (m + 1)], Wv[:, 128 * m:128 * (m + 1)]],
                axis=1)),
            "wo": bf(Wo[512 * m:512 * (m + 1), :]),
            "vkT": np.ascontiguousarray(vk[m].T),
            "vvT": np.ascontiguousarray(vv[m].T),
            "lkA": lkA, "lkB": lkBs, "lvA": lvA, "lvB": lvBs,
            "qw": np.ascontiguousarray(qnw[:, None]),
            "kw": np.ascontiguousarray(knw[:, None]),
            "cosT": cosT, "sinT": sinT, "mtri": mtri,
            "rotm": rotm, "ident": ident, "onesc": onesc, "onesr": onesr,
            "onesel": onesel, "onesel2": onesel2,
        })

    nc = _get_nc()
    res = run_bass_kernel_spmd(nc, in_maps, core_ids=list(range(8)))
    acc = res.results[0]["out"].astype(np.float32)
    for m in range(1, 8):
        acc = acc + res.results[m]["out"].astype(np.float32)
    return acc[None]  # (1, T, D)
